# revision 2
# baseline (speedup 1.0000x reference)
"""Sparse (sliding-window) GQA attention prefill kernel for 8 Trainium2 cores.

Sharding: TP=4 over KV heads (2 KV heads + 10 Q heads per core) x DP=2 over
batch. Core c: batch = c // 4, shard q4 = c % 4.

Device program (SPMD, identical on all cores; per-core data via inputs):
  A1: xqT/xkT projections in transposed layout ([head_dim, seq]); sum-of-squares
      for the global RMS norm via Square + ones-matmul; tiny AllReduce of the
      norm partials within each batch group.
  A2: V projection in natural layout ([seq, head_dim]); rope tables scaled by
      the norm factors (r_q * scale folded into cos/sin tables).
  B:  per (head, 2-query-tile group): scoresT = K^T-chunk.T @ qT in the sliding
      band, mask add, exp (no max subtraction -- logits are bounded), ones-matmul
      denominator, P^T @ V accumulation, divide by denominator on evacuation.
  C:  AllGather of attnT (bf16) within batch group; out = attnT.T @ wo col-shard.
"""

import sys
import numpy as np

for _p in ("/opt/trn_rl_repo", "/root/.axon_site/_ro/trn_rl_repo"):
    if _p not in sys.path:
        sys.path.insert(0, _p)

import ml_dtypes

import concourse.bass as bass
import concourse.tile as tile
from concourse import bacc, mybir
from concourse import bass_utils

F32 = mybir.dt.float32
BF16 = mybir.dt.bfloat16
BF16_NP = ml_dtypes.bfloat16
AF = mybir.ActivationFunctionType


class Cfg:
    def __init__(self, S=2048, DIM=5120, HQ=40, HKV=8, TP=4, DP=2, SW=1024,
                 MSCALE=1.2079441541679836, EPS=1e-6):
        self.S, self.DIM, self.HQ, self.HKV = S, DIM, HQ, HKV
        self.TP, self.DP, self.SW = TP, DP, SW
        self.MSCALE, self.EPS = MSCALE, EPS
        self.D = 128
        self.NC = TP * DP
        self.HQL = HQ // TP          # local q heads
        self.KVL = HKV // TP         # local kv heads
        self.REP = HQ // HKV
        self.KC = DIM // 128         # contraction chunks
        self.NT = S // 128           # seq tiles
        self.G = self.NT // 2        # 2-query-tile groups
        self.WD = SW // 128          # window in tiles
        self.COLS = DIM // TP        # output column shard
        self.HCL = self.HQL + self.KVL  # projection chains with transposed out
        self.NST = S // 512          # 512-wide s-tiles (phase A1)
        self.NST2 = S // 256         # 256-wide s-tiles (phase A2)
        self.CQ = self.D ** -0.5 * MSCALE
        assert self.WD >= 2 and self.NT > self.WD + 1 and self.NT % 2 == 0
        self.groups = [[b * TP + r for r in range(TP)] for b in range(DP)]


def head_chunks(C):
    """Per-kv head pair chunks: [(kv, [h0,h1]), (kv, [h2,h3]), (kv, [h4])...]"""
    out = []
    per = C.HQL // C.KVL
    for kv in range(C.KVL):
        hs = list(range(kv * per, (kv + 1) * per))
        i = 0
        while i < len(hs):
            out.append((kv, hs[i:i + 2]))
            i += 2
    return out


def attention_tile_kernel(tc, C, io):
    nc = tc.nc
    S, KC, HQL, KVL, NT, G, WD = C.S, C.KC, C.HQL, C.KVL, C.NT, C.G, C.WD
    xT16, wqkv, wv_in, wo_in = io["xT16"], io["wqkv"], io["wv_in"], io["wo_in"]
    cosT_in, sinT_in, masks_in, wnorm_in = io["cosT"], io["sinT"], io["masks"], io["wnorm"]
    out_sh = io["out_sh"]
    chunks = head_chunks(C)

    from contextlib import ExitStack
    ctx = ExitStack()
    with ctx:
        singles = ctx.enter_context(tc.tile_pool(name="singles", bufs=1))
        dramcc = ctx.enter_context(tc.tile_pool(name="dramcc", bufs=1, space="DRAM"))

        ones16 = singles.tile([128, 1], BF16)
        nc.vector.memset(ones16[:], 1.0)
        wnorm_sb = singles.tile([128, HQL + KVL], F32)
        nc.sync.dma_start(wnorm_sb[:], wnorm_in[:])

        cc_nin = dramcc.tile([1, 2 * S], F32)
        cc_nout = dramcc.tile([1, 2 * S], F32)
        cc_ains = [dramcc.tile([len(hs), 128, S], BF16, name=f"ccai{ci}")
                   for ci, (kv, hs) in enumerate(chunks)]
        cc_aouts = [dramcc.tile([C.TP, len(hs), 128, S], BF16, name=f"ccao{ci}")
                    for ci, (kv, hs) in enumerate(chunks)]

        with (
            tc.tile_pool(name="xqp", bufs=1) as xq_pool,
            tc.tile_pool(name="xkp", bufs=1) as xk_pool,
            tc.tile_pool(name="vp", bufs=1) as v_pool,
        ):
            xq_sb = xq_pool.tile([128, HQL, S], BF16)
            xk_sb = xk_pool.tile([128, KVL, S], BF16)

            # ---- phase A1: q/k projections (transposed out) + norm partials --
            with (
                tc.tile_pool(name="xt1", bufs=2) as xt1,
                tc.tile_pool(name="wst", bufs=3) as wst,
                tc.tile_pool(name="sqp", bufs=2) as sqp,
                tc.tile_pool(name="trow", bufs=2) as trow,
                tc.tile_pool(name="psA", bufs=3, space="PSUM") as psA,
                tc.tile_pool(name="psN", bufs=1, space="PSUM") as psN,
            ):
                for st in range(C.NST):
                    s0 = st * 512
                    xt = xt1.tile([128, KC, 512], BF16, tag="xt")
                    nc.sync.dma_start(
                        xt[:],
                        xT16[:, :, s0:s0 + 512].rearrange("kc p s -> p kc s"))
                    ps_nq = psN.tile([1, 512], F32, tag="nq")
                    ps_nk = psN.tile([1, 512], F32, tag="nk")
                    for hc in range(C.HCL):
                        w_sb = wst.tile([128, KC, 128], BF16, tag="w")
                        nc.sync.dma_start(w_sb[:], wqkv[hc])
                        ps = psA.tile([128, 512], F32, tag="proj")
                        for kc in range(KC):
                            nc.tensor.matmul(ps[:], w_sb[:, kc, :], xt[:, kc, :],
                                             start=(kc == 0), stop=(kc == KC - 1))
                        if hc < HQL:
                            dest = xq_sb[:, hc, s0:s0 + 512]
                        else:
                            dest = xk_sb[:, hc - HQL, s0:s0 + 512]
                        nc.vector.tensor_scalar_mul(dest, ps[:],
                                                    wnorm_sb[:, hc:hc + 1])
                        sq = sqp.tile([128, 512], BF16, tag="sq")
                        nc.scalar.activation(sq[:], ps[:], AF.Square)
                        tgt = ps_nq if hc < HQL else ps_nk
                        first = (hc == 0) or (hc == HQL)
                        last = (hc == HQL - 1) or (hc == C.HCL - 1)
                        nc.tensor.matmul(tgt[:], ones16[:], sq[:],
                                         start=first, stop=last)
                    rq_t = trow.tile([1, 512], F32, tag="rq")
                    rk_t = trow.tile([1, 512], F32, tag="rk")
                    nc.vector.tensor_copy(rq_t[:], ps_nq[:])
                    nc.vector.tensor_copy(rk_t[:], ps_nk[:])
                    nc.sync.dma_start(cc_nin[0:1, s0:s0 + 512], rq_t[:])
                    nc.sync.dma_start(cc_nin[0:1, S + s0:S + s0 + 512], rk_t[:])

            # runs on the collective engine during phase A2
            nc.gpsimd.collective_compute(
                "AllReduce", mybir.AluOpType.add, replica_groups=C.groups,
                ins=[cc_nin.opt()], outs=[cc_nout.opt()])

            # ---- phase A2: V projection (natural layout) --------------------
            v_sb = v_pool.tile([128, NT, KVL, 128], BF16)
            with (
                tc.tile_pool(name="xt2", bufs=2) as xt2,
                tc.tile_pool(name="wvp", bufs=1) as wvp,
                tc.tile_pool(name="psV", bufs=3, space="PSUM") as psV,
            ):
                wv_sb = wvp.tile([128, KC, KVL * 128], BF16)
                nc.sync.dma_start(wv_sb[:], wv_in[:])
                for st in range(C.NST2):
                    s0 = st * 256
                    xt = xt2.tile([128, KC, 256], BF16, tag="xt2")
                    nc.sync.dma_start(
                        xt[:],
                        xT16[:, :, s0:s0 + 256].rearrange("kc p s -> p kc s"))
                    for tc4 in range(2):
                        tt = 2 * st + tc4
                        psv = psV.tile([128, KVL * 128], F32, tag="v")
                        for kc in range(KC):
                            nc.tensor.matmul(
                                psv[:], xt[:, kc, tc4 * 128:(tc4 + 1) * 128],
                                wv_sb[:, kc, :],
                                start=(kc == 0), stop=(kc == KC - 1))
                        nc.vector.tensor_copy(v_sb[:, tt, :, :], psv[:])

            # ---- norm rows + rope tables (overlaps A2 tail) -----------------
            with tc.tile_pool(name="tabsQ", bufs=1) as tabs_q:
                cosq = tabs_q.tile([128, S], F32)
                sinq = tabs_q.tile([128, S], F32)
                cosk = tabs_q.tile([128, S], F32)
                sink = tabs_q.tile([128, S], F32)

                with (
                    tc.tile_pool(name="rowsp", bufs=1) as rowsp,
                    tc.tile_pool(name="tabsO", bufs=1) as tabs_orig,
                ):
                    rq_row = rowsp.tile([1, S], F32)
                    rk_row = rowsp.tile([1, S], F32)
                    nc.sync.dma_start(rq_row[:], cc_nout[0:1, 0:S])
                    nc.sync.dma_start(rk_row[:], cc_nout[0:1, S:2 * S])
                    cq2 = C.CQ * C.CQ
                    nc.vector.tensor_scalar(
                        rq_row[:], rq_row[:],
                        scalar1=1.0 / (C.DIM * cq2), scalar2=C.EPS / cq2,
                        op0=mybir.AluOpType.mult, op1=mybir.AluOpType.add)
                    nc.scalar.activation(rq_row[:], rq_row[:], AF.Sqrt)
                    nc.vector.reciprocal(rq_row[:], rq_row[:])
                    nc.vector.tensor_scalar(
                        rk_row[:], rk_row[:],
                        scalar1=1.0 / (C.HKV * 128), scalar2=C.EPS,
                        op0=mybir.AluOpType.mult, op1=mybir.AluOpType.add)
                    nc.scalar.activation(rk_row[:], rk_row[:], AF.Sqrt)
                    nc.vector.reciprocal(rk_row[:], rk_row[:])

                    rq_b = rowsp.tile([128, S], F32)
                    rk_b = rowsp.tile([128, S], F32)
                    nc.gpsimd.partition_broadcast(rq_b[:], rq_row[:])
                    nc.gpsimd.partition_broadcast(rk_b[:], rk_row[:])

                    cosT = tabs_orig.tile([128, S], F32)
                    sinT = tabs_orig.tile([128, S], F32)
                    nc.sync.dma_start(cosT[:], cosT_in[:])
                    nc.sync.dma_start(sinT[:], sinT_in[:])
                    nc.vector.tensor_mul(cosq[:], cosT[:], rq_b[:])
                    nc.vector.tensor_mul(sinq[:], sinT[:], rq_b[:])
                    nc.vector.tensor_mul(cosk[:], cosT[:], rk_b[:])
                    nc.vector.tensor_mul(sink[:], sinT[:], rk_b[:])

                # ---- rope + phase B (paired heads) --------------------------
                def rope_inplace(src, cos_sb, sin_sb, rp):
                    for r0 in range(0, S, 512):
                        sl = slice(r0, r0 + 512)
                        rot = rp.tile([128, 512], BF16, tag="rot")
                        nc.sync.dma_start(rot[0:64, :], src[64:128, sl])
                        nc.sync.dma_start(rot[64:128, :], src[0:64, sl])
                        t1 = rp.tile([128, 512], F32, tag="t1")
                        t2 = rp.tile([128, 512], F32, tag="t2")
                        nc.vector.tensor_mul(t1[:], src[:, sl], cos_sb[:, sl])
                        nc.vector.tensor_mul(t2[:], rot[:], sin_sb[:, sl])
                        nc.vector.tensor_add(src[:, sl], t1[:], t2[:])

                with (
                    tc.tile_pool(name="attnp", bufs=1) as attnp,
                    tc.tile_pool(name="ropep", bufs=2) as ropep,
                    tc.tile_pool(name="maskp", bufs=1) as maskp,
                    tc.tile_pool(name="expp", bufs=6) as expp,
                    tc.tile_pool(name="bmisc", bufs=3) as bmisc,
                    tc.tile_pool(name="psSC", bufs=4, space="PSUM") as psSC,
                    tc.tile_pool(name="psAT", bufs=2, space="PSUM") as psAT,
                    tc.tile_pool(name="psDN", bufs=2, space="PSUM") as psDN,
                ):
                    attnT = attnp.tile([128, HQL, S], BF16)
                    masks_sb = maskp.tile([128, 4, 256], F32)
                    nc.sync.dma_start(masks_sb[:], masks_in[:])
                    for kv in range(KVL):
                        rope_inplace(xk_sb[:, kv, :], cosk, sink, ropep)
                    off2m = {0: 0, 1: 1, WD: 2, WD + 1: 3}
                    for ci, (kv, hs) in enumerate(chunks):
                        w = len(hs)
                        h0 = hs[0]
                        for h in hs:
                            rope_inplace(xq_sb[:, h, :], cosq, sinq, ropep)
                        for g in range(G):
                            jlo, jhi = max(0, 2 * g - WD), 2 * g + 1
                            ps_at = psAT.tile([128, 512], F32, tag="at")
                            ps_dn = psDN.tile([1, 512], F32, tag="dn")
                            for j in range(jlo, jhi + 1):
                                ps_sc = psSC.tile([128, 512], F32, tag="sc")
                                nc.tensor.matmul(
                                    ps_sc[:, :w * 256].rearrange(
                                        "p (w s) -> p w s", w=w),
                                    xk_sb[:, kv, j * 128:(j + 1) * 128],
                                    xq_sb[:, h0:h0 + w, g * 256:(g + 1) * 256],
                                    start=True, stop=True)
                                m = off2m.get(jhi - j)
                                if m is not None:
                                    nc.vector.tensor_add(
                                        ps_sc[:, :w * 256].rearrange(
                                            "p (w s) -> p w s", w=w),
                                        ps_sc[:, :w * 256].rearrange(
                                            "p (w s) -> p w s", w=w),
                                        masks_sb[:, m, None, :].to_broadcast(
                                            (128, w, 256)))
                                ex = expp.tile([128, 512], BF16, tag="ex")
                                nc.scalar.activation(ex[:, :w * 256],
                                                     ps_sc[:, :w * 256], AF.Exp)
                                nc.tensor.matmul(ps_dn[:, :w * 256], ones16[:],
                                                 ex[:, :w * 256],
                                                 start=(j == jlo), stop=(j == jhi))
                                nc.tensor.matmul(ps_at[:, :w * 256],
                                                 v_sb[:, j, kv, :],
                                                 ex[:, :w * 256],
                                                 start=(j == jlo), stop=(j == jhi))
                            den_r = bmisc.tile([1, 512], F32, tag="denr")
                            nc.vector.reciprocal(den_r[:, :w * 256],
                                                 ps_dn[:, :w * 256])
                            den_rb = bmisc.tile([128, 512], F32, tag="denrb")
                            nc.gpsimd.partition_broadcast(den_rb[:, :w * 256],
                                                          den_r[:, :w * 256])
                            nc.vector.tensor_mul(
                                attnT[:, h0:h0 + w, g * 256:(g + 1) * 256],
                                ps_at[:, :w * 256].rearrange(
                                    "p (w s) -> p w s", w=w),
                                den_rb[:, :w * 256].rearrange(
                                    "p (w s) -> p w s", w=w))
                        # chunk finished -> ship its attnT slab + AllGather
                        nc.sync.dma_start(
                            cc_ains[ci].rearrange("h p s -> p h s"),
                            attnT[:, h0:h0 + w, :])
                        nc.gpsimd.collective_compute(
                            "AllGather", mybir.AluOpType.bypass,
                            replica_groups=C.groups,
                            ins=[cc_ains[ci].opt()], outs=[cc_aouts[ci].opt()])

        # ---- phase C: output projection from gathered attnT ------------------
        with (
            tc.tile_pool(name="wop", bufs=1) as wop,
            tc.tile_pool(name="lhsp", bufs=2) as lhsp,
            tc.tile_pool(name="outp", bufs=2) as outp,
            tc.tile_pool(name="psO", bufs=3, space="PSUM") as psO,
        ):
            wo_sb = wop.tile([128, C.HQ, C.COLS], BF16)
            nc.sync.dma_start(wo_sb[:], wo_in[:])
            # slot -> global head mapping, chunk-major (gather completion order)
            slot_map = []
            base = 0
            chunk_base = []
            for ci, (kv, hs) in enumerate(chunks):
                chunk_base.append(base)
                for r in range(C.TP):
                    for hl in hs:
                        slot_map.append(r * HQL + hl)
                base += C.TP * len(hs)
            col_ts = []
            c0 = 0
            while c0 < C.COLS:
                wdt = min(512, C.COLS - c0)
                col_ts.append((c0, wdt))
                c0 += wdt
            for sb in range(NT):
                lhs = lhsp.tile([128, C.HQ, 128], BF16, tag="lhs")
                for ci, (kv, hs) in enumerate(chunks):
                    nslots = C.TP * len(hs)
                    nc.sync.dma_start(
                        lhs[:, chunk_base[ci]:chunk_base[ci] + nslots, :],
                        cc_aouts[ci][:, :, :, sb * 128:(sb + 1) * 128].rearrange(
                            "r h p s -> p (r h) s"))
                orow = outp.tile([128, C.COLS], F32, tag="orow")
                for (c0, wdt) in col_ts:
                    ps_o = psO.tile([128, 512], F32, tag="o")
                    for slot in range(C.HQ):
                        gh = slot_map[slot]
                        nc.tensor.matmul(ps_o[:, :wdt], lhs[:, slot, :],
                                         wo_sb[:, gh, c0:c0 + wdt],
                                         start=(slot == 0), stop=(slot == C.HQ - 1))
                    nc.vector.tensor_copy(orow[:, c0:c0 + wdt], ps_o[:, :wdt])
                nc.sync.dma_start(out_sh[sb * 128:(sb + 1) * 128, :], orow[:])


def build_program(C):
    nc = bacc.Bacc("TRN2", target_bir_lowering=False, debug=False,
                   num_devices=C.NC)
    io = {
        "xT16": nc.dram_tensor("xT16", [C.KC, 128, C.S], BF16, kind="ExternalInput").ap(),
        "wqkv": nc.dram_tensor("wqkv", [C.HCL, 128, C.KC, 128], BF16,
                               kind="ExternalInput").ap(),
        "wv_in": nc.dram_tensor("wv_in", [128, C.KC, C.KVL * 128], BF16,
                                kind="ExternalInput").ap(),
        "wo_in": nc.dram_tensor("wo_in", [128, C.HQ, C.COLS], BF16,
                                kind="ExternalInput").ap(),
        "cosT": nc.dram_tensor("cosT", [128, C.S], F32, kind="ExternalInput").ap(),
        "sinT": nc.dram_tensor("sinT", [128, C.S], F32, kind="ExternalInput").ap(),
        "masks": nc.dram_tensor("masks", [128, 4, 256], F32, kind="ExternalInput").ap(),
        "wnorm": nc.dram_tensor("wnorm", [128, C.HQL + C.KVL], F32,
                                kind="ExternalInput").ap(),
        "out_sh": nc.dram_tensor("out_sh", [C.S, C.COLS], F32,
                                 kind="ExternalOutput").ap(),
    }
    with tile.TileContext(nc) as tc:
        attention_tile_kernel(tc, C, io)
    nc.compile()
    return nc


def make_masks(mask_np, C):
    """4 mask tiles [t,s-pair] for offsets {0,1,WD,WD+1}; returns [128,4,256] f32."""
    S, WD, SW = C.S, C.WD, C.SW
    I0 = WD + 1

    def tileT(d):
        i, j = I0, I0 - d
        if 0 <= j < C.NT:
            blk = np.array(mask_np[i * 128:(i + 1) * 128, j * 128:(j + 1) * 128],
                           dtype=np.float64)
        else:
            blk = np.full((128, 128), -np.inf)
        s_idx = np.arange(128)[:, None]
        t_idx = np.arange(128)[None, :]
        dist = 128 * d + s_idx - t_idx
        blk = np.where(dist > SW, -np.inf, blk)
        return np.maximum(blk.T, -1e30).astype(np.float32)   # [t, s]

    tiles = []
    for off in (0, 1, WD, WD + 1):
        dl, dr = off - 1, off
        tiles.append(np.concatenate([tileT(dl), tileT(dr)], axis=1))
    return np.ascontiguousarray(np.stack(tiles, axis=1))      # [128, 4, 256]


def make_core_inputs(inputs, C):
    x = np.asarray(inputs["x"], dtype=np.float32)
    wq = np.asarray(inputs["wq"], dtype=np.float32)
    wk = np.asarray(inputs["wk"], dtype=np.float32)
    wv = np.asarray(inputs["wv"], dtype=np.float32)
    wo = np.asarray(inputs["wo"], dtype=np.float32)
    qw = np.asarray(inputs["q_norm_weight"], dtype=np.float32)
    kw = np.asarray(inputs["k_norm_weight"], dtype=np.float32)
    ch = np.asarray(inputs["cos_half"], dtype=np.float32)
    sh = np.asarray(inputs["sin_half"], dtype=np.float32)
    mask = np.asarray(inputs["mask"], dtype=np.float32)
    assert int(inputs.get("start_pos", 0) or 0) == 0

    cosT = np.ascontiguousarray(np.concatenate([ch.T, ch.T], axis=0))
    sinT = np.ascontiguousarray(np.concatenate([-sh.T, sh.T], axis=0))
    masks = make_masks(mask, C)
    KC, HQL, KVL = C.KC, C.HQL, C.KVL

    xT_cache = {}
    for b in range(C.DP):
        xT_cache[b] = np.ascontiguousarray(x[b].T).astype(BF16_NP).reshape(
            C.KC, 128, C.S)
    in_maps = []
    for c in range(C.NC):
        b, q4 = c // C.TP, c % C.TP
        x16 = xT_cache[b]
        wq_s = wq[:, 128 * HQL * q4:128 * HQL * (q4 + 1)]
        wk_s = wk[:, 128 * KVL * q4:128 * KVL * (q4 + 1)]
        wv_s = wv[:, 128 * KVL * q4:128 * KVL * (q4 + 1)]
        wqk = np.concatenate([wq_s, wk_s], axis=1).astype(BF16_NP)
        # [HCL, 128, KC, 128]: per chain, contraction-partition-major
        wqkv_pre = np.ascontiguousarray(
            wqk.reshape(KC, 128, C.HCL, 128).transpose(2, 1, 0, 3))
        wv_pre = np.ascontiguousarray(
            wv_s.astype(BF16_NP).reshape(KC, 128, KVL * 128).transpose(1, 0, 2))
        wo_s = wo[:, C.COLS * q4:C.COLS * (q4 + 1)].astype(BF16_NP)
        wo_pre = np.ascontiguousarray(
            wo_s.reshape(C.HQ, 128, C.COLS).transpose(1, 0, 2))
        wn = np.zeros((128, HQL + KVL), dtype=np.float32)
        for hc in range(HQL):
            g = HQL * q4 + hc
            wn[:, hc] = qw[128 * g:128 * (g + 1)]
        for j in range(KVL):
            g = KVL * q4 + j
            wn[:, HQL + j] = kw[128 * g:128 * (g + 1)]
        in_maps.append({"xT16": x16, "wqkv": wqkv_pre, "wv_in": wv_pre,
                        "wo_in": wo_pre, "cosT": cosT, "sinT": sinT,
                        "masks": masks, "wnorm": wn})
    return in_maps


_CACHED = {}


def run(inputs, C=None, trace=False, stitch=None, trace_cores=None):
    C = C or Cfg()
    key = (C.S, C.DIM, C.HQ, C.HKV, C.TP, C.DP, C.SW)
    if key not in _CACHED:
        _CACHED[key] = build_program(C)
    nc = _CACHED[key]
    in_maps = make_core_inputs(inputs, C)
    if stitch is None:
        stitch = trace
    if trace and trace_cores is None:
        trace_cores = list(range(C.NC))
    res = bass_utils.run_bass_kernel_spmd(
        nc, in_maps, core_ids=list(range(C.NC)), trace=trace,
        stitch_traces=stitch, trace_cores=trace_cores if trace else None)
    out = np.empty((C.DP, C.S, C.DIM), dtype=np.float32)
    for c in range(C.NC):
        b, q4 = c // C.TP, c % C.TP
        out[b, :, C.COLS * q4:C.COLS * (q4 + 1)] = res.results[c]["out_sh"]
    return out, res


def kernel(**inputs) -> np.ndarray:
    out, _ = run(inputs)
    return out



# revision 9
# speedup vs baseline: 1.0171x; 1.0171x over previous
"""Sparse (sliding-window) GQA attention prefill kernel for 8 Trainium2 cores.

Sharding: TP=4 over KV heads (2 KV heads + 10 Q heads per core) x DP=2 over
batch. Core c: batch = c // 4, shard q4 = c % 4.

Device program (SPMD, identical on all cores; per-core data via inputs):
  A1: xqT/xkT projections in transposed layout ([head_dim, seq]); sum-of-squares
      for the global RMS norm via Square + ones-matmul; per-s-tile AllReduce of
      the norm partials within each batch group (overlaps A1 compute).
  A2: V projection in natural layout ([seq, head_dim]).
  R:  norm rows -> rope tables (bf16, norm factor folded in); batched rope over
      all local heads per 512-slice (vector for q, gpsimd for k), overlapping A2.
  B:  per (head-pair chunk, 2-query-tile group): scoresT = K^T-chunk.T @ qT in
      the sliding band, mask add, exp, ones-matmul denominator, P^T @ V
      accumulation. Software-pipelined: the denominator/PV matmuls for step j
      are emitted after the scores matmul of step j+1 so the PE never waits on
      the exp. Divide on evacuation via broadcast + reciprocal on [128, .].
      attnT shipped in s-halves: AllGather per (chunk, half), issued late so the
      collective's input wait never blocks the gpsimd queue.
  C:  out = attnT.T @ wo col-shard; first s-half rows start as soon as the
      half-0 gathers land.
"""

import sys
import numpy as np

for _p in ("/opt/trn_rl_repo", "/root/.axon_site/_ro/trn_rl_repo"):
    if _p not in sys.path:
        sys.path.insert(0, _p)

import ml_dtypes

import concourse.bass as bass
import concourse.tile as tile
from concourse import bacc, mybir
from concourse import bass_utils

F32 = mybir.dt.float32
BF16 = mybir.dt.bfloat16
BF16_NP = ml_dtypes.bfloat16
AF = mybir.ActivationFunctionType
ALU = mybir.AluOpType


class Cfg:
    def __init__(self, S=2048, DIM=5120, HQ=40, HKV=8, TP=4, DP=2, SW=1024,
                 MSCALE=1.2079441541679836, EPS=1e-6):
        self.S, self.DIM, self.HQ, self.HKV = S, DIM, HQ, HKV
        self.TP, self.DP, self.SW = TP, DP, SW
        self.MSCALE, self.EPS = MSCALE, EPS
        self.D = 128
        self.NC = TP * DP
        self.HQL = HQ // TP          # local q heads
        self.KVL = HKV // TP         # local kv heads
        self.REP = HQ // HKV
        self.KC = DIM // 128         # contraction chunks
        self.NT = S // 128           # seq tiles
        self.G = self.NT // 2        # 2-query-tile groups
        self.WD = SW // 128          # window in tiles
        self.COLS = DIM // TP        # output column shard
        self.HCL = self.HQL + self.KVL  # projection chains with transposed out
        self.NST = S // 512          # 512-wide s-tiles (phase A1)
        self.NST2 = S // 256         # 256-wide s-tiles (phase A2)
        self.CQ = self.D ** -0.5 * MSCALE
        assert self.WD >= 2 and self.NT > self.WD + 1 and self.NT % 2 == 0
        self.groups = [[b * TP + r for r in range(TP)] for b in range(DP)]


def head_chunks(C):
    """Per-kv head pair chunks: [(kv, [h0,h1]), (kv, [h2,h3]), (kv, [h4])...]"""
    out = []
    per = C.HQL // C.KVL
    for kv in range(C.KVL):
        hs = list(range(kv * per, (kv + 1) * per))
        i = 0
        while i < len(hs):
            out.append((kv, hs[i:i + 2]))
            i += 2
    return out


def attention_tile_kernel(tc, C, io):
    nc = tc.nc
    S, KC, HQL, KVL, NT, G, WD = C.S, C.KC, C.HQL, C.KVL, C.NT, C.G, C.WD
    H2 = S // 2
    xT16, wqkv, wv_in, wo_in = io["xT16"], io["wqkv"], io["wv_in"], io["wo_in"]
    cosT_in, sinT_in, masks_in, wnorm_in = io["cosT"], io["sinT"], io["masks"], io["wnorm"]
    out_sh = io["out_sh"]
    chunks = head_chunks(C)

    from contextlib import ExitStack
    ctx = ExitStack()
    with ctx:
        singles = ctx.enter_context(tc.tile_pool(name="singles", bufs=1))
        dramcc = ctx.enter_context(tc.tile_pool(name="dramcc", bufs=1, space="DRAM"))

        ones16 = singles.tile([128, 1], BF16)
        nc.vector.memset(ones16[:], 1.0)
        wnorm_sb = singles.tile([128, HQL + KVL], F32)
        nc.sync.dma_start(wnorm_sb[:], wnorm_in[:])

        cc_nins = [dramcc.tile([1, 1024], F32, name=f"ccni{st}")
                   for st in range(C.NST)]
        cc_nouts = [dramcc.tile([1, 1024], F32, name=f"ccno{st}")
                    for st in range(C.NST)]
        cc_ains = [[dramcc.tile([len(hs), 128, H2], BF16, name=f"ccai{ci}_{hf}")
                    for hf in range(2)] for ci, (kv, hs) in enumerate(chunks)]
        cc_aouts = [[dramcc.tile([C.TP, len(hs), 128, H2], BF16,
                                 name=f"ccao{ci}_{hf}") for hf in range(2)]
                    for ci, (kv, hs) in enumerate(chunks)]

        with (
            tc.tile_pool(name="xqp", bufs=1) as xq_pool,
            tc.tile_pool(name="xkp", bufs=1) as xk_pool,
            tc.tile_pool(name="vp", bufs=1) as v_pool,
        ):
            xq_sb = xq_pool.tile([128, HQL, S], BF16)
            xk_sb = xk_pool.tile([128, KVL, S], BF16)

            # ---- phase A1: q/k projections (transposed out) + norm partials --
            KCH = KC // 2
            with (
                tc.tile_pool(name="xt1", bufs=2) as xt1,
                tc.tile_pool(name="wst", bufs=3) as wst,
                tc.tile_pool(name="sqp", bufs=2) as sqp,
                tc.tile_pool(name="trow", bufs=2) as trow,
                tc.tile_pool(name="psA", bufs=3, space="PSUM") as psA,
                tc.tile_pool(name="psN", bufs=1, space="PSUM") as psN,
            ):
                for st in range(C.NST):
                    s0 = st * 512
                    xt_a = xt1.tile([128, KCH, 512], BF16, tag="xta")
                    xt_b = xt1.tile([128, KCH, 512], BF16, tag="xtb")
                    nc.sync.dma_start(
                        xt_a[:],
                        xT16[:KCH, :, s0:s0 + 512].rearrange("kc p s -> p kc s"))
                    nc.sync.dma_start(
                        xt_b[:],
                        xT16[KCH:, :, s0:s0 + 512].rearrange("kc p s -> p kc s"))
                    ps_nq = psN.tile([1, 512], F32, tag="nq")
                    ps_nk = psN.tile([1, 512], F32, tag="nk")
                    for hc in range(C.HCL):
                        w_sb = wst.tile([128, KC, 128], BF16, tag="w")
                        nc.sync.dma_start(w_sb[:], wqkv[hc])
                        ps = psA.tile([128, 512], F32, tag="proj")
                        for kc in range(KC):
                            xsrc = xt_a if kc < KCH else xt_b
                            nc.tensor.matmul(ps[:], w_sb[:, kc, :],
                                             xsrc[:, kc % KCH, :],
                                             start=(kc == 0), stop=(kc == KC - 1))
                        if hc < HQL:
                            dest = xq_sb[:, hc, s0:s0 + 512]
                        else:
                            dest = xk_sb[:, hc - HQL, s0:s0 + 512]
                        nc.vector.tensor_scalar_mul(dest, ps[:],
                                                    wnorm_sb[:, hc:hc + 1])
                        sq = sqp.tile([128, 512], BF16, tag="sq")
                        nc.scalar.activation(sq[:], ps[:], AF.Square)
                        tgt = ps_nq if hc < HQL else ps_nk
                        first = (hc == 0) or (hc == HQL)
                        last = (hc == HQL - 1) or (hc == C.HCL - 1)
                        nc.tensor.matmul(tgt[:], ones16[:], sq[:],
                                         start=first, stop=last)
                    rq_t = trow.tile([1, 512], F32, tag="rq")
                    rk_t = trow.tile([1, 512], F32, tag="rk")
                    nc.vector.tensor_copy(rq_t[:], ps_nq[:])
                    nc.vector.tensor_copy(rk_t[:], ps_nk[:])
                    nc.sync.dma_start(cc_nins[st][0:1, 0:512], rq_t[:])
                    nc.sync.dma_start(cc_nins[st][0:1, 512:1024], rk_t[:])
                    # per-s-tile AllReduce of norm partials: overlaps A1 compute
                    nc.gpsimd.collective_compute(
                        "AllReduce", ALU.add, replica_groups=C.groups,
                        ins=[cc_nins[st].opt()], outs=[cc_nouts[st].opt()])

            # ---- phases R + A2, interleaved emission -------------------------
            # A2's first DMAs are prefetched, then R (norm rows + rope tables +
            # batched rope on vector/gpsimd/scalar) is emitted so it overlaps
            # A2's PE matmuls. A2 evacuates through the scalar engine so R owns
            # the vector queue. R keys off the per-st AllReduces from A1.
            v_sb = v_pool.tile([128, NT, KVL, 128], BF16)
            with (
                tc.tile_pool(name="xt2", bufs=2) as xt2,
                tc.tile_pool(name="wvp", bufs=1) as wvp,
                tc.tile_pool(name="psV", bufs=3, space="PSUM") as psV,
                tc.tile_pool(name="tabsQ", bufs=1) as tabs_q,
                tc.tile_pool(name="rowsp", bufs=1) as rowsp,
                tc.tile_pool(name="ropep", bufs=2) as ropep,
            ):
                wv_sb = wvp.tile([128, KC, KVL * 128], BF16)
                nc.sync.dma_start(wv_sb[:], wv_in[:])
                xts = {}
                for st in range(2):
                    s0 = st * 256
                    xts[st] = xt2.tile([128, KC, 256], BF16, tag="xt2",
                                       name=f"xtp{st}")
                    nc.sync.dma_start(
                        xts[st][:],
                        xT16[:, :, s0:s0 + 256].rearrange("kc p s -> p kc s"))

                # -- phase R emission --
                cq2 = C.CQ * C.CQ
                cosq = tabs_q.tile([128, S], BF16)
                sinq = tabs_q.tile([128, S], BF16)
                cosk = tabs_q.tile([128, S], BF16)
                sink = tabs_q.tile([128, S], BF16)

                nraw = rowsp.tile([1, S], F32)
                kraw = rowsp.tile([1, S], F32)
                for st in range(C.NST):
                    nc.sync.dma_start(nraw[0:1, st * 512:(st + 1) * 512],
                                      cc_nouts[st][0:1, 0:512])
                    nc.sync.dma_start(kraw[0:1, st * 512:(st + 1) * 512],
                                      cc_nouts[st][0:1, 512:1024])
                rq_b = rowsp.tile([128, S], F32)
                rk_b = rowsp.tile([128, S], F32)
                nc.gpsimd.partition_broadcast(rk_b[:], kraw[:])
                nc.gpsimd.partition_broadcast(rq_b[:], nraw[:])
                # rq_b = 1/sqrt(raw/(DIM*cq2) + eps/cq2)  (CQ scale folded in)
                nc.gpsimd.tensor_scalar(
                    rk_b[:], rk_b[:],
                    scalar1=1.0 / (C.HKV * 128), scalar2=C.EPS,
                    op0=ALU.mult, op1=ALU.add)
                nc.scalar.activation(rk_b[:], rk_b[:], AF.Sqrt)
                nc.vector.reciprocal(rk_b[:], rk_b[:])
                nc.gpsimd.tensor_scalar(
                    rq_b[:], rq_b[:],
                    scalar1=1.0 / (C.DIM * cq2), scalar2=C.EPS / cq2,
                    op0=ALU.mult, op1=ALU.add)
                nc.scalar.activation(rq_b[:], rq_b[:], AF.Sqrt)
                nc.vector.reciprocal(rq_b[:], rq_b[:])

                cosT = rowsp.tile([128, S], BF16)
                sinT = rowsp.tile([128, S], BF16)
                nc.sync.dma_start(cosT[:], cosT_in[:])
                nc.sync.dma_start(sinT[:], sinT_in[:])
                nc.vector.tensor_mul(cosk[:], cosT[:], rk_b[:])
                nc.vector.tensor_mul(sink[:], sinT[:], rk_b[:])
                nc.vector.tensor_mul(cosq[:], cosT[:], rq_b[:])
                nc.vector.tensor_mul(sinq[:], sinT[:], rq_b[:])

                # batched in-place rope per 512-slice: k heads on gpsimd
                # (ready first for chunk 0), q heads on vector.
                for st in range(C.NST):
                    sl = slice(st * 512, (st + 1) * 512)
                    rotk = ropep.tile([128, KVL, 512], BF16, tag="rotk")
                    rotq = ropep.tile([128, HQL, 512], BF16, tag="rotq")
                    nc.sync.dma_start(rotk[0:64], xk_sb[64:128, :, sl])
                    nc.sync.dma_start(rotk[64:128], xk_sb[0:64, :, sl])
                    nc.sync.dma_start(rotq[0:64], xq_sb[64:128, :, sl])
                    nc.sync.dma_start(rotq[64:128], xq_sb[0:64, :, sl])
                    # fully in-place: rot *= sin; x *= cos; x += rot
                    nc.gpsimd.tensor_mul(
                        rotk[:], rotk[:],
                        sink[:, None, sl].to_broadcast((128, KVL, 512)))
                    nc.gpsimd.tensor_mul(
                        xk_sb[:, :, sl], xk_sb[:, :, sl],
                        cosk[:, None, sl].to_broadcast((128, KVL, 512)))
                    nc.gpsimd.tensor_add(xk_sb[:, :, sl], xk_sb[:, :, sl],
                                         rotk[:])
                    nc.vector.tensor_mul(
                        rotq[:], rotq[:],
                        sinq[:, None, sl].to_broadcast((128, HQL, 512)))
                    nc.vector.tensor_mul(
                        xq_sb[:, :, sl], xq_sb[:, :, sl],
                        cosq[:, None, sl].to_broadcast((128, HQL, 512)))
                    nc.vector.tensor_add(xq_sb[:, :, sl], xq_sb[:, :, sl],
                                         rotq[:])

                # -- phase A2 matmul loop --
                for st in range(C.NST2):
                    s0 = st * 256
                    if st in xts:
                        xt = xts.pop(st)
                    else:
                        xt = xt2.tile([128, KC, 256], BF16, tag="xt2")
                        nc.sync.dma_start(
                            xt[:],
                            xT16[:, :, s0:s0 + 256].rearrange("kc p s -> p kc s"))
                    for tc4 in range(2):
                        tt = 2 * st + tc4
                        psv = psV.tile([128, KVL * 128], F32, tag="v")
                        for kc in range(KC):
                            nc.tensor.matmul(
                                psv[:], xt[:, kc, tc4 * 128:(tc4 + 1) * 128],
                                wv_sb[:, kc, :],
                                start=(kc == 0), stop=(kc == KC - 1))
                        nc.scalar.copy(v_sb[:, tt, :, :], psv[:])

            # ---- phase B: banded attention, software-pipelined --------------
            with (
                tc.tile_pool(name="attnp", bufs=1) as attnp,
                tc.tile_pool(name="maskp", bufs=1) as maskp,
                tc.tile_pool(name="expp", bufs=6) as expp,
                tc.tile_pool(name="bmisc", bufs=3) as bmisc,
                tc.tile_pool(name="psSC", bufs=4, space="PSUM") as psSC,
                tc.tile_pool(name="psAT", bufs=2, space="PSUM") as psAT,
                tc.tile_pool(name="psDN", bufs=2, space="PSUM") as psDN,
            ):
                attnT = attnp.tile([128, HQL, S], BF16)
                masks_sb = maskp.tile([128, 4, 256], F32)
                nc.sync.dma_start(masks_sb[:], masks_in[:])
                off2m = {0: 0, 1: 1, WD: 2, WD + 1: 3}
                # gathers whose issue we delay so the collective's input wait
                # doesn't stall the gpsimd queue ahead of broadcasts
                pend_cc = []

                def flush_cc():
                    while pend_cc:
                        ci_, hf_ = pend_cc.pop(0)
                        nc.gpsimd.collective_compute(
                            "AllGather", ALU.bypass,
                            replica_groups=C.groups,
                            ins=[cc_ains[ci_][hf_].opt()],
                            outs=[cc_aouts[ci_][hf_].opt()])

                for ci, (kv, hs) in enumerate(chunks):
                    w = len(hs)
                    h0 = hs[0]
                    for g in range(G):
                        jlo, jhi = max(0, 2 * g - WD), 2 * g + 1
                        ps_at = psAT.tile([128, 512], F32, tag="at")
                        ps_dn = psDN.tile([1, 512], F32, tag="dn")
                        pend = None  # (ex tile, j) waiting for dn/at emission
                        for j in range(jlo, jhi + 1):
                            ps_sc = psSC.tile([128, 512], F32, tag="sc")
                            nc.tensor.matmul(
                                ps_sc[:, :w * 256].rearrange(
                                    "p (w s) -> p w s", w=w),
                                xk_sb[:, kv, j * 128:(j + 1) * 128],
                                xq_sb[:, h0:h0 + w, g * 256:(g + 1) * 256],
                                start=True, stop=True)
                            if pend is not None:
                                exp_, jp = pend
                                nc.tensor.matmul(ps_dn[:, :w * 256], ones16[:],
                                                 exp_[:, :w * 256],
                                                 start=(jp == jlo), stop=False)
                                nc.tensor.matmul(ps_at[:, :w * 256],
                                                 v_sb[:, jp, kv, :],
                                                 exp_[:, :w * 256],
                                                 start=(jp == jlo), stop=False)
                            m = off2m.get(jhi - j)
                            if m is not None:
                                nc.vector.tensor_add(
                                    ps_sc[:, :w * 256].rearrange(
                                        "p (w s) -> p w s", w=w),
                                    ps_sc[:, :w * 256].rearrange(
                                        "p (w s) -> p w s", w=w),
                                    masks_sb[:, m, None, :].to_broadcast(
                                        (128, w, 256)))
                            ex = expp.tile([128, 512], BF16, tag="ex")
                            nc.scalar.activation(ex[:, :w * 256],
                                                 ps_sc[:, :w * 256], AF.Exp)
                            pend = (ex, j)
                        exp_, jp = pend
                        nc.tensor.matmul(ps_dn[:, :w * 256], ones16[:],
                                         exp_[:, :w * 256],
                                         start=(jp == jlo), stop=True)
                        nc.tensor.matmul(ps_at[:, :w * 256],
                                         v_sb[:, jp, kv, :],
                                         exp_[:, :w * 256],
                                         start=(jp == jlo), stop=True)
                        dn_sb = bmisc.tile([1, 512], F32, tag="dnsb")
                        nc.vector.tensor_copy(dn_sb[:, :w * 256],
                                              ps_dn[:, :w * 256])
                        den_b = bmisc.tile([128, 512], F32, tag="denb")
                        nc.gpsimd.partition_broadcast(den_b[:, :w * 256],
                                                      dn_sb[:, :w * 256])
                        nc.vector.reciprocal(den_b[:, :w * 256],
                                             den_b[:, :w * 256])
                        nc.vector.tensor_mul(
                            attnT[:, h0:h0 + w, g * 256:(g + 1) * 256],
                            ps_at[:, :w * 256].rearrange(
                                "p (w s) -> p w s", w=w),
                            den_b[:, :w * 256].rearrange(
                                "p (w s) -> p w s", w=w))
                        if g == G // 2 - 1 or g == G - 1:
                            hf = 0 if g == G // 2 - 1 else 1
                            nc.sync.dma_start(
                                cc_ains[ci][hf].rearrange("h p s -> p h s"),
                                attnT[:, h0:h0 + w, hf * H2:(hf + 1) * H2])
                            pend_cc.append((ci, hf))
                        if g == 0 or g == G // 2:
                            flush_cc()
                flush_cc()

        # ---- phase C: output projection from gathered attnT ------------------
        with (
            tc.tile_pool(name="wop", bufs=1) as wop,
            tc.tile_pool(name="lhsp", bufs=2) as lhsp,
            tc.tile_pool(name="outp", bufs=2) as outp,
            tc.tile_pool(name="psO", bufs=3, space="PSUM") as psO,
        ):
            wo_sb = wop.tile([128, C.HQ, C.COLS], BF16)
            nc.sync.dma_start(wo_sb[:], wo_in[:])
            # slot -> global head mapping, chunk-major (gather completion order)
            slot_map = []
            base = 0
            chunk_base = []
            for ci, (kv, hs) in enumerate(chunks):
                chunk_base.append(base)
                for r in range(C.TP):
                    for hl in hs:
                        slot_map.append(r * HQL + hl)
                base += C.TP * len(hs)
            col_ts = []
            c0 = 0
            while c0 < C.COLS:
                wdt = min(512, C.COLS - c0)
                col_ts.append((c0, wdt))
                c0 += wdt
            for sb in range(NT):
                hf, sbr = sb // (NT // 2), sb % (NT // 2)
                lhs = lhsp.tile([128, C.HQ, 128], BF16, tag="lhs")
                for ci, (kv, hs) in enumerate(chunks):
                    nslots = C.TP * len(hs)
                    nc.sync.dma_start(
                        lhs[:, chunk_base[ci]:chunk_base[ci] + nslots, :],
                        cc_aouts[ci][hf][
                            :, :, :, sbr * 128:(sbr + 1) * 128].rearrange(
                            "r h p s -> p (r h) s"))
                orow = outp.tile([128, C.COLS], F32, tag="orow")
                for (c0, wdt) in col_ts:
                    ps_o = psO.tile([128, 512], F32, tag="o")
                    for slot in range(C.HQ):
                        gh = slot_map[slot]
                        nc.tensor.matmul(ps_o[:, :wdt], lhs[:, slot, :],
                                         wo_sb[:, gh, c0:c0 + wdt],
                                         start=(slot == 0), stop=(slot == C.HQ - 1))
                    nc.vector.tensor_copy(orow[:, c0:c0 + wdt], ps_o[:, :wdt])
                nc.sync.dma_start(out_sh[sb * 128:(sb + 1) * 128, :], orow[:])


def build_program(C):
    nc = bacc.Bacc("TRN2", target_bir_lowering=False, debug=False,
                   num_devices=C.NC)
    io = {
        "xT16": nc.dram_tensor("xT16", [C.KC, 128, C.S], BF16, kind="ExternalInput").ap(),
        "wqkv": nc.dram_tensor("wqkv", [C.HCL, 128, C.KC, 128], BF16,
                               kind="ExternalInput").ap(),
        "wv_in": nc.dram_tensor("wv_in", [128, C.KC, C.KVL * 128], BF16,
                                kind="ExternalInput").ap(),
        "wo_in": nc.dram_tensor("wo_in", [128, C.HQ, C.COLS], BF16,
                                kind="ExternalInput").ap(),
        "cosT": nc.dram_tensor("cosT", [128, C.S], BF16, kind="ExternalInput").ap(),
        "sinT": nc.dram_tensor("sinT", [128, C.S], BF16, kind="ExternalInput").ap(),
        "masks": nc.dram_tensor("masks", [128, 4, 256], F32, kind="ExternalInput").ap(),
        "wnorm": nc.dram_tensor("wnorm", [128, C.HQL + C.KVL], F32,
                                kind="ExternalInput").ap(),
        "out_sh": nc.dram_tensor("out_sh", [C.S, C.COLS], F32,
                                 kind="ExternalOutput").ap(),
    }
    with tile.TileContext(nc) as tc:
        attention_tile_kernel(tc, C, io)
    nc.compile()
    return nc


def make_masks(mask_np, C):
    """4 mask tiles [t,s-pair] for offsets {0,1,WD,WD+1}; returns [128,4,256] f32."""
    S, WD, SW = C.S, C.WD, C.SW
    I0 = WD + 1

    def tileT(d):
        i, j = I0, I0 - d
        if 0 <= j < C.NT:
            blk = np.array(mask_np[i * 128:(i + 1) * 128, j * 128:(j + 1) * 128],
                           dtype=np.float64)
        else:
            blk = np.full((128, 128), -np.inf)
        s_idx = np.arange(128)[:, None]
        t_idx = np.arange(128)[None, :]
        dist = 128 * d + s_idx - t_idx
        blk = np.where(dist > SW, -np.inf, blk)
        return np.maximum(blk.T, -1e30).astype(np.float32)   # [t, s]

    tiles = []
    for off in (0, 1, WD, WD + 1):
        dl, dr = off - 1, off
        tiles.append(np.concatenate([tileT(dl), tileT(dr)], axis=1))
    return np.ascontiguousarray(np.stack(tiles, axis=1))      # [128, 4, 256]


def make_core_inputs(inputs, C):
    x = np.asarray(inputs["x"], dtype=np.float32)
    wq = np.asarray(inputs["wq"], dtype=np.float32)
    wk = np.asarray(inputs["wk"], dtype=np.float32)
    wv = np.asarray(inputs["wv"], dtype=np.float32)
    wo = np.asarray(inputs["wo"], dtype=np.float32)
    qw = np.asarray(inputs["q_norm_weight"], dtype=np.float32)
    kw = np.asarray(inputs["k_norm_weight"], dtype=np.float32)
    ch = np.asarray(inputs["cos_half"], dtype=np.float32)
    sh = np.asarray(inputs["sin_half"], dtype=np.float32)
    mask = np.asarray(inputs["mask"], dtype=np.float32)
    assert int(inputs.get("start_pos", 0) or 0) == 0

    cosT = np.ascontiguousarray(
        np.concatenate([ch.T, ch.T], axis=0)).astype(BF16_NP)
    sinT = np.ascontiguousarray(
        np.concatenate([-sh.T, sh.T], axis=0)).astype(BF16_NP)
    masks = make_masks(mask, C)
    KC, HQL, KVL = C.KC, C.HQL, C.KVL

    xT_cache = {}
    for b in range(C.DP):
        xT_cache[b] = np.ascontiguousarray(x[b].T).astype(BF16_NP).reshape(
            C.KC, 128, C.S)
    in_maps = []
    for c in range(C.NC):
        b, q4 = c // C.TP, c % C.TP
        x16 = xT_cache[b]
        wq_s = wq[:, 128 * HQL * q4:128 * HQL * (q4 + 1)]
        wk_s = wk[:, 128 * KVL * q4:128 * KVL * (q4 + 1)]
        wv_s = wv[:, 128 * KVL * q4:128 * KVL * (q4 + 1)]
        wqk = np.concatenate([wq_s, wk_s], axis=1).astype(BF16_NP)
        # [HCL, 128, KC, 128]: per chain, contraction-partition-major
        wqkv_pre = np.ascontiguousarray(
            wqk.reshape(KC, 128, C.HCL, 128).transpose(2, 1, 0, 3))
        wv_pre = np.ascontiguousarray(
            wv_s.astype(BF16_NP).reshape(KC, 128, KVL * 128).transpose(1, 0, 2))
        wo_s = wo[:, C.COLS * q4:C.COLS * (q4 + 1)].astype(BF16_NP)
        wo_pre = np.ascontiguousarray(
            wo_s.reshape(C.HQ, 128, C.COLS).transpose(1, 0, 2))
        wn = np.zeros((128, HQL + KVL), dtype=np.float32)
        for hc in range(HQL):
            g = HQL * q4 + hc
            wn[:, hc] = qw[128 * g:128 * (g + 1)]
        for j in range(KVL):
            g = KVL * q4 + j
            wn[:, HQL + j] = kw[128 * g:128 * (g + 1)]
        in_maps.append({"xT16": x16, "wqkv": wqkv_pre, "wv_in": wv_pre,
                        "wo_in": wo_pre, "cosT": cosT, "sinT": sinT,
                        "masks": masks, "wnorm": wn})
    return in_maps


_CACHED = {}


def run(inputs, C=None, trace=False, stitch=None, trace_cores=None):
    C = C or Cfg()
    key = (C.S, C.DIM, C.HQ, C.HKV, C.TP, C.DP, C.SW)
    if key not in _CACHED:
        _CACHED[key] = build_program(C)
    nc = _CACHED[key]
    in_maps = make_core_inputs(inputs, C)
    if stitch is None:
        stitch = trace
    if trace and trace_cores is None:
        trace_cores = list(range(C.NC))
    res = bass_utils.run_bass_kernel_spmd(
        nc, in_maps, core_ids=list(range(C.NC)), trace=trace,
        stitch_traces=stitch, trace_cores=trace_cores if trace else None)
    out = np.empty((C.DP, C.S, C.DIM), dtype=np.float32)
    for c in range(C.NC):
        b, q4 = c // C.TP, c % C.TP
        out[b, :, C.COLS * q4:C.COLS * (q4 + 1)] = res.results[c]["out_sh"]
    return out, res


def kernel(**inputs) -> np.ndarray:
    out, _ = run(inputs)
    return out


# revision 17
# speedup vs baseline: 1.0830x; 1.0648x over previous
"""Sparse (sliding-window) GQA attention prefill kernel for 8 Trainium2 cores.

Sharding: TP=4 over KV heads (2 KV heads + 10 Q heads per core) x DP=2 over
batch. Core c: batch = c // 4, shard q4 = c % 4.

Device program (SPMD, identical on all cores; per-core data via inputs):
  A1: xqT/xkT projections in transposed layout ([head_dim, seq]); sum-of-squares
      for the global RMS norm via Square + ones-matmul; per-s-tile AllReduce of
      the norm partials within each batch group (overlaps A1 compute).
  A2: V projection in natural layout ([seq, head_dim]).
  R:  norm rows -> rope tables (bf16, norm factor folded in); batched rope over
      all local heads per 512-slice (vector for q, gpsimd for k), overlapping A2.
  B:  per (head-pair chunk, 2-query-tile group): scoresT = K^T-chunk.T @ qT in
      the sliding band, mask add, exp, ones-matmul denominator, P^T @ V
      accumulation. Software-pipelined: the denominator/PV matmuls for step j
      are emitted after the scores matmul of step j+1 so the PE never waits on
      the exp. Divide on evacuation via broadcast + reciprocal on [128, .].
      attnT shipped in s-halves: AllGather per (chunk, half), issued late so the
      collective's input wait never blocks the gpsimd queue.
  C:  out = attnT.T @ wo col-shard; first s-half rows start as soon as the
      half-0 gathers land.
"""

import sys
import numpy as np

for _p in ("/opt/trn_rl_repo", "/root/.axon_site/_ro/trn_rl_repo"):
    if _p not in sys.path:
        sys.path.insert(0, _p)

import ml_dtypes

import concourse.bass as bass
import concourse.tile as tile
from concourse import bacc, mybir
from concourse import bass_utils

F32 = mybir.dt.float32
BF16 = mybir.dt.bfloat16
BF16_NP = ml_dtypes.bfloat16
AF = mybir.ActivationFunctionType
ALU = mybir.AluOpType


class Cfg:
    def __init__(self, S=2048, DIM=5120, HQ=40, HKV=8, TP=4, DP=2, SW=1024,
                 MSCALE=1.2079441541679836, EPS=1e-6):
        self.S, self.DIM, self.HQ, self.HKV = S, DIM, HQ, HKV
        self.TP, self.DP, self.SW = TP, DP, SW
        self.MSCALE, self.EPS = MSCALE, EPS
        self.D = 128
        self.NC = TP * DP
        self.HQL = HQ // TP          # local q heads
        self.KVL = HKV // TP         # local kv heads
        self.REP = HQ // HKV
        self.KC = DIM // 128         # contraction chunks
        self.NT = S // 128           # seq tiles
        self.G = self.NT // 2        # 2-query-tile groups
        self.WD = SW // 128          # window in tiles
        self.COLS = DIM // TP        # output column shard
        self.HCL = self.HQL + self.KVL  # projection chains with transposed out
        self.NST = S // 512          # 512-wide s-tiles (phase A1)
        self.NST2 = S // 256         # 256-wide s-tiles (phase A2)
        self.CQ = self.D ** -0.5 * MSCALE
        assert self.WD >= 2 and self.NT > self.WD + 1 and self.NT % 2 == 0
        self.groups = [[b * TP + r for r in range(TP)] for b in range(DP)]


def head_chunks(C):
    """Per-kv head pair chunks: [(kv, [h0,h1]), (kv, [h2,h3]), (kv, [h4])...]"""
    out = []
    per = C.HQL // C.KVL
    for kv in range(C.KVL):
        hs = list(range(kv * per, (kv + 1) * per))
        i = 0
        while i < len(hs):
            out.append((kv, hs[i:i + 2]))
            i += 2
    return out


def attention_tile_kernel(tc, C, io):
    nc = tc.nc
    S, KC, HQL, KVL, NT, G, WD = C.S, C.KC, C.HQL, C.KVL, C.NT, C.G, C.WD
    H2 = S // 2
    xT16, wqkv, wv_in, wo_in = io["xT16"], io["wqkv"], io["wv_in"], io["wo_in"]
    cosT_in, sinT_in, masks_in, wnorm_in = io["cosT"], io["sinT"], io["masks"], io["wnorm"]
    out_sh = io["out_sh"]
    chunks = head_chunks(C)

    from contextlib import ExitStack
    ctx = ExitStack()
    with ctx:
        singles = ctx.enter_context(tc.tile_pool(name="singles", bufs=1))
        dramcc = ctx.enter_context(tc.tile_pool(name="dramcc", bufs=1, space="DRAM"))

        ones16 = singles.tile([128, 1], BF16)
        nc.vector.memset(ones16[:], 1.0)
        wnorm_sb = singles.tile([128, HQL + KVL], F32)
        nc.sync.dma_start(wnorm_sb[:], wnorm_in[:])

        cc_nins = [dramcc.tile([1, 1024], F32, name=f"ccni{st}")
                   for st in range(C.NST)]
        cc_nouts = [dramcc.tile([1, 1024], F32, name=f"ccno{st}")
                    for st in range(C.NST)]
        cc_ains = [[dramcc.tile([len(hs), 128, H2], BF16, name=f"ccai{ci}_{hf}")
                    for hf in range(2)] for ci, (kv, hs) in enumerate(chunks)]
        cc_aouts = [[dramcc.tile([C.TP, len(hs), 128, H2], BF16,
                                 name=f"ccao{ci}_{hf}") for hf in range(2)]
                    for ci, (kv, hs) in enumerate(chunks)]

        with (
            tc.tile_pool(name="xqp", bufs=1) as xq_pool,
            tc.tile_pool(name="xkp", bufs=1) as xk_pool,
            tc.tile_pool(name="vp", bufs=1) as v_pool,
        ):
            xq_sb = xq_pool.tile([128, HQL, S], BF16)
            xk_sb = xk_pool.tile([128, KVL, S], BF16)

            # ---- phase A1: q/k projections (transposed out) + norm partials --
            KCH = KC // 2
            with (
                tc.tile_pool(name="xt1", bufs=2) as xt1,
                tc.tile_pool(name="wst", bufs=3) as wst,
                tc.tile_pool(name="sqp", bufs=2) as sqp,
                tc.tile_pool(name="trow", bufs=2) as trow,
                tc.tile_pool(name="psA", bufs=3, space="PSUM") as psA,
                tc.tile_pool(name="psN", bufs=1, space="PSUM") as psN,
            ):
                for st in range(C.NST):
                    s0 = st * 512
                    xt_a = xt1.tile([128, KCH, 512], BF16, tag="xta")
                    xt_b = xt1.tile([128, KCH, 512], BF16, tag="xtb")
                    nc.sync.dma_start(
                        xt_a[:],
                        xT16[:KCH, :, s0:s0 + 512].rearrange("kc p s -> p kc s"))
                    nc.sync.dma_start(
                        xt_b[:],
                        xT16[KCH:, :, s0:s0 + 512].rearrange("kc p s -> p kc s"))
                    ps_nq = psN.tile([1, 512], F32, tag="nq")
                    ps_nk = psN.tile([1, 512], F32, tag="nk")
                    for hc in range(C.HCL):
                        w_sb = wst.tile([128, KC, 128], BF16, tag="w")
                        nc.sync.dma_start(w_sb[:], wqkv[hc])
                        ps = psA.tile([128, 512], F32, tag="proj")
                        for kc in range(KC):
                            xsrc = xt_a if kc < KCH else xt_b
                            nc.tensor.matmul(ps[:], w_sb[:, kc, :],
                                             xsrc[:, kc % KCH, :],
                                             start=(kc == 0), stop=(kc == KC - 1))
                        if hc < HQL:
                            dest = xq_sb[:, hc, s0:s0 + 512]
                        else:
                            dest = xk_sb[:, hc - HQL, s0:s0 + 512]
                        nc.vector.tensor_scalar_mul(dest, ps[:],
                                                    wnorm_sb[:, hc:hc + 1])
                        sq = sqp.tile([128, 512], BF16, tag="sq")
                        nc.scalar.activation(sq[:], ps[:], AF.Square)
                        tgt = ps_nq if hc < HQL else ps_nk
                        first = (hc == 0) or (hc == HQL)
                        last = (hc == HQL - 1) or (hc == C.HCL - 1)
                        nc.tensor.matmul(tgt[:], ones16[:], sq[:],
                                         start=first, stop=last)
                    rq_t = trow.tile([1, 512], F32, tag="rq")
                    rk_t = trow.tile([1, 512], F32, tag="rk")
                    nc.vector.tensor_copy(rq_t[:], ps_nq[:])
                    nc.vector.tensor_copy(rk_t[:], ps_nk[:])
                    nc.sync.dma_start(cc_nins[st][0:1, 0:512], rq_t[:])
                    nc.sync.dma_start(cc_nins[st][0:1, 512:1024], rk_t[:])
                    # per-s-tile AllReduce of norm partials: overlaps A1 compute
                    nc.gpsimd.collective_compute(
                        "AllReduce", ALU.add, replica_groups=C.groups,
                        ins=[cc_nins[st].opt()], outs=[cc_nouts[st].opt()])

            # ---- phases R + A2, interleaved emission -------------------------
            # A2's first DMAs are prefetched, then R (norm rows + rope tables +
            # batched rope on vector/gpsimd/scalar) is emitted so it overlaps
            # A2's PE matmuls. A2 evacuates through the scalar engine so R owns
            # the vector queue. R keys off the per-st AllReduces from A1.
            v_sb = v_pool.tile([128, NT, KVL, 128], BF16)
            with (
                tc.tile_pool(name="xt2", bufs=3) as xt2,
                tc.tile_pool(name="wvp", bufs=1) as wvp,
                tc.tile_pool(name="psV", bufs=3, space="PSUM") as psV,
                tc.tile_pool(name="tabsQ", bufs=1) as tabs_q,
                tc.tile_pool(name="rowsp", bufs=1) as rowsp,
                tc.tile_pool(name="ropep", bufs=2) as ropep,
            ):
                wv_sb = wvp.tile([128, KC, KVL * 128], BF16)
                nc.sync.dma_start(wv_sb[:], wv_in[:])
                xts = {}
                for st in range(3):
                    s0 = st * 256
                    xts[st] = xt2.tile([128, KC, 256], BF16, tag="xt2",
                                       name=f"xtp{st}")
                    nc.sync.dma_start(
                        xts[st][:],
                        xT16[:, :, s0:s0 + 256].rearrange("kc p s -> p kc s"))

                # -- phase R emission --
                cq2 = C.CQ * C.CQ
                cosq = tabs_q.tile([128, S], BF16)
                sinq = tabs_q.tile([128, S], BF16)
                cosk = tabs_q.tile([128, S], BF16)
                sink = tabs_q.tile([128, S], BF16)

                nraw = rowsp.tile([1, S], F32)
                kraw = rowsp.tile([1, S], F32)
                for st in range(C.NST):
                    nc.sync.dma_start(nraw[0:1, st * 512:(st + 1) * 512],
                                      cc_nouts[st][0:1, 0:512])
                    nc.sync.dma_start(kraw[0:1, st * 512:(st + 1) * 512],
                                      cc_nouts[st][0:1, 512:1024])
                rq_b = rowsp.tile([128, S], F32)
                rk_b = rowsp.tile([128, S], F32)
                nc.gpsimd.partition_broadcast(rk_b[:], kraw[:])
                nc.gpsimd.partition_broadcast(rq_b[:], nraw[:])
                # rq_b = 1/sqrt(raw/(DIM*cq2) + eps/cq2)  (CQ scale folded in)
                nc.gpsimd.tensor_scalar(
                    rk_b[:], rk_b[:],
                    scalar1=1.0 / (C.HKV * 128), scalar2=C.EPS,
                    op0=ALU.mult, op1=ALU.add)
                nc.scalar.activation(rk_b[:], rk_b[:], AF.Sqrt)
                nc.vector.reciprocal_approx_fast(out=rk_b[:], in_=rk_b[:])
                nc.gpsimd.tensor_scalar(
                    rq_b[:], rq_b[:],
                    scalar1=1.0 / (C.DIM * cq2), scalar2=C.EPS / cq2,
                    op0=ALU.mult, op1=ALU.add)
                nc.scalar.activation(rq_b[:], rq_b[:], AF.Sqrt)
                nc.vector.reciprocal_approx_fast(out=rq_b[:], in_=rq_b[:])

                nc.sync.dma_start(cosk[:], cosT_in[:])
                nc.sync.dma_start(sink[:], sinT_in[:])
                nc.sync.dma_start(cosq[:], cosT_in[:])
                nc.sync.dma_start(sinq[:], sinT_in[:])
                nc.gpsimd.tensor_mul(cosk[:], cosk[:], rk_b[:])
                nc.gpsimd.tensor_mul(sink[:], sink[:], rk_b[:])
                nc.vector.tensor_mul(cosq[:], cosq[:], rq_b[:])
                nc.vector.tensor_mul(sinq[:], sinq[:], rq_b[:])

                # batched in-place rope per 512-slice: k heads + first q half
                # on gpsimd (ready first for chunk 0), rest of q on vector.
                QH = HQL // 2
                for st in range(C.NST):
                    sl = slice(st * 512, (st + 1) * 512)
                    rotk = ropep.tile([128, KVL, 512], BF16, tag="rotk",
                                      bufs=1)
                    rotq = ropep.tile([128, HQL, 512], BF16, tag="rotq")
                    nc.sync.dma_start(rotk[0:64], xk_sb[64:128, :, sl])
                    nc.sync.dma_start(rotk[64:128], xk_sb[0:64, :, sl])
                    nc.sync.dma_start(rotq[0:64], xq_sb[64:128, :, sl])
                    nc.sync.dma_start(rotq[64:128], xq_sb[0:64, :, sl])
                    # fully in-place: rot *= sin; x *= cos; x += rot
                    nc.gpsimd.tensor_mul(
                        rotk[:], rotk[:],
                        sink[:, None, sl].to_broadcast((128, KVL, 512)))
                    nc.gpsimd.tensor_mul(
                        xk_sb[:, :, sl], xk_sb[:, :, sl],
                        cosk[:, None, sl].to_broadcast((128, KVL, 512)))
                    nc.gpsimd.tensor_add(xk_sb[:, :, sl], xk_sb[:, :, sl],
                                         rotk[:])
                    for eng, hsl in ((nc.gpsimd, slice(0, QH)),
                                     (nc.vector, slice(QH, HQL))):
                        nh = hsl.stop - hsl.start
                        eng.tensor_mul(
                            rotq[:, hsl, :], rotq[:, hsl, :],
                            sinq[:, None, sl].to_broadcast((128, nh, 512)))
                        eng.tensor_mul(
                            xq_sb[:, hsl, sl], xq_sb[:, hsl, sl],
                            cosq[:, None, sl].to_broadcast((128, nh, 512)))
                        eng.tensor_add(xq_sb[:, hsl, sl], xq_sb[:, hsl, sl],
                                       rotq[:, hsl, :])

                # -- phase A2 matmul loop --
                for st in range(C.NST2):
                    s0 = st * 256
                    if st in xts:
                        xt = xts.pop(st)
                    else:
                        xt = xt2.tile([128, KC, 256], BF16, tag="xt2")
                        nc.sync.dma_start(
                            xt[:],
                            xT16[:, :, s0:s0 + 256].rearrange("kc p s -> p kc s"))
                    for tc4 in range(2):
                        tt = 2 * st + tc4
                        psv = psV.tile([128, KVL * 128], F32, tag="v")
                        for kc in range(KC):
                            nc.tensor.matmul(
                                psv[:], xt[:, kc, tc4 * 128:(tc4 + 1) * 128],
                                wv_sb[:, kc, :],
                                start=(kc == 0), stop=(kc == KC - 1))
                        nc.scalar.copy(v_sb[:, tt, :, :], psv[:])

            # ---- phase B: banded attention, software-pipelined --------------
            with (
                tc.tile_pool(name="attnp", bufs=1) as attnp,
                tc.tile_pool(name="maskp", bufs=1) as maskp,
                tc.tile_pool(name="expp", bufs=6) as expp,
                tc.tile_pool(name="bmisc", bufs=3) as bmisc,
                tc.tile_pool(name="psSC", bufs=4, space="PSUM") as psSC,
                tc.tile_pool(name="psAT", bufs=2, space="PSUM") as psAT,
                tc.tile_pool(name="psDN", bufs=2, space="PSUM") as psDN,
            ):
                attnT = attnp.tile([128, HQL, S], BF16)
                masks_sb = maskp.tile([128, 4, 256], F32)
                nc.sync.dma_start(masks_sb[:], masks_in[:])
                off2m = {0: 0, 1: 1, WD: 2, WD + 1: 3}
                # gathers whose issue we delay so the collective's input wait
                # doesn't stall the gpsimd queue ahead of broadcasts
                pend_cc = []

                def flush_cc():
                    while pend_cc:
                        ci_, hf_ = pend_cc.pop(0)
                        nc.gpsimd.collective_compute(
                            "AllGather", ALU.bypass,
                            replica_groups=C.groups,
                            ins=[cc_ains[ci_][hf_].opt()],
                            outs=[cc_aouts[ci_][hf_].opt()])

                for ci, (kv, hs) in enumerate(chunks):
                    w = len(hs)
                    h0 = hs[0]
                    for g in range(G):
                        jlo, jhi = max(0, 2 * g - WD), 2 * g + 1
                        ps_at = psAT.tile([128, 512], F32, tag="at")
                        ps_dn = psDN.tile([1, 512], F32, tag="dn")
                        pend = None  # (ex tile, j) waiting for dn/at emission
                        for j in range(jlo, jhi + 1):
                            ps_sc = psSC.tile([128, 512], F32, tag="sc")
                            nc.tensor.matmul(
                                ps_sc[:, :w * 256].rearrange(
                                    "p (w s) -> p w s", w=w),
                                xk_sb[:, kv, j * 128:(j + 1) * 128],
                                xq_sb[:, h0:h0 + w, g * 256:(g + 1) * 256],
                                start=True, stop=True)
                            if pend is not None:
                                exp_, jp = pend
                                nc.tensor.matmul(ps_dn[:, :w * 256], ones16[:],
                                                 exp_[:, :w * 256],
                                                 start=(jp == jlo), stop=False)
                                nc.tensor.matmul(ps_at[:, :w * 256],
                                                 v_sb[:, jp, kv, :],
                                                 exp_[:, :w * 256],
                                                 start=(jp == jlo), stop=False)
                            m = off2m.get(jhi - j)
                            if m is not None:
                                nc.vector.tensor_add(
                                    ps_sc[:, :w * 256].rearrange(
                                        "p (w s) -> p w s", w=w),
                                    ps_sc[:, :w * 256].rearrange(
                                        "p (w s) -> p w s", w=w),
                                    masks_sb[:, m, None, :].to_broadcast(
                                        (128, w, 256)))
                            ex = expp.tile([128, 512], BF16, tag="ex")
                            nc.scalar.activation(ex[:, :w * 256],
                                                 ps_sc[:, :w * 256], AF.Exp)
                            pend = (ex, j)
                        exp_, jp = pend
                        nc.tensor.matmul(ps_dn[:, :w * 256], ones16[:],
                                         exp_[:, :w * 256],
                                         start=(jp == jlo), stop=True)
                        nc.tensor.matmul(ps_at[:, :w * 256],
                                         v_sb[:, jp, kv, :],
                                         exp_[:, :w * 256],
                                         start=(jp == jlo), stop=True)
                        dn_sb = bmisc.tile([1, 512], F32, tag="dnsb")
                        nc.vector.tensor_copy(dn_sb[:, :w * 256],
                                              ps_dn[:, :w * 256])
                        den_b = bmisc.tile([128, 512], F32, tag="denb")
                        nc.gpsimd.partition_broadcast(den_b[:, :w * 256],
                                                      dn_sb[:, :w * 256])
                        nc.vector.reciprocal_approx_fast(
                            out=den_b[:, :w * 256], in_=den_b[:, :w * 256])
                        nc.vector.tensor_mul(
                            attnT[:, h0:h0 + w, g * 256:(g + 1) * 256],
                            ps_at[:, :w * 256].rearrange(
                                "p (w s) -> p w s", w=w),
                            den_b[:, :w * 256].rearrange(
                                "p (w s) -> p w s", w=w))
                        if g == G // 2 - 1 or g == G - 1:
                            hf = 0 if g == G // 2 - 1 else 1
                            nc.sync.dma_start(
                                cc_ains[ci][hf].rearrange("h p s -> p h s"),
                                attnT[:, h0:h0 + w, hf * H2:(hf + 1) * H2])
                            pend_cc.append((ci, hf))
                        if g == 0 or g == G // 2:
                            flush_cc()
                flush_cc()

        # ---- phase C: output projection from gathered attnT ------------------
        with (
            tc.tile_pool(name="wop", bufs=1) as wop,
            tc.tile_pool(name="lhsp", bufs=2) as lhsp,
            tc.tile_pool(name="outp", bufs=2) as outp,
            tc.tile_pool(name="psO", bufs=3, space="PSUM") as psO,
        ):
            wo_sb = wop.tile([128, C.HQ, C.COLS], BF16)
            nc.sync.dma_start(wo_sb[:], wo_in[:])
            # slot -> global head mapping, chunk-major (gather completion order)
            slot_map = []
            base = 0
            chunk_base = []
            for ci, (kv, hs) in enumerate(chunks):
                chunk_base.append(base)
                for r in range(C.TP):
                    for hl in hs:
                        slot_map.append(r * HQL + hl)
                base += C.TP * len(hs)
            col_ts = []
            c0 = 0
            while c0 < C.COLS:
                wdt = min(512, C.COLS - c0)
                col_ts.append((c0, wdt))
                c0 += wdt
            for sb in range(NT):
                hf, sbr = sb // (NT // 2), sb % (NT // 2)
                lhs = lhsp.tile([128, C.HQ, 128], BF16, tag="lhs")
                for ci, (kv, hs) in enumerate(chunks):
                    nslots = C.TP * len(hs)
                    nc.sync.dma_start(
                        lhs[:, chunk_base[ci]:chunk_base[ci] + nslots, :],
                        cc_aouts[ci][hf][
                            :, :, :, sbr * 128:(sbr + 1) * 128].rearrange(
                            "r h p s -> p (r h) s"))
                orow = outp.tile([128, C.COLS], F32, tag="orow")
                for (c0, wdt) in col_ts:
                    ps_o = psO.tile([128, 512], F32, tag="o")
                    for slot in range(C.HQ):
                        gh = slot_map[slot]
                        nc.tensor.matmul(ps_o[:, :wdt], lhs[:, slot, :],
                                         wo_sb[:, gh, c0:c0 + wdt],
                                         start=(slot == 0), stop=(slot == C.HQ - 1))
                    nc.vector.tensor_copy(orow[:, c0:c0 + wdt], ps_o[:, :wdt])
                nc.sync.dma_start(out_sh[sb * 128:(sb + 1) * 128, :], orow[:])


def build_program(C):
    nc = bacc.Bacc("TRN2", target_bir_lowering=False, debug=False,
                   num_devices=C.NC)
    io = {
        "xT16": nc.dram_tensor("xT16", [C.KC, 128, C.S], BF16, kind="ExternalInput").ap(),
        "wqkv": nc.dram_tensor("wqkv", [C.HCL, 128, C.KC, 128], BF16,
                               kind="ExternalInput").ap(),
        "wv_in": nc.dram_tensor("wv_in", [128, C.KC, C.KVL * 128], BF16,
                                kind="ExternalInput").ap(),
        "wo_in": nc.dram_tensor("wo_in", [128, C.HQ, C.COLS], BF16,
                                kind="ExternalInput").ap(),
        "cosT": nc.dram_tensor("cosT", [128, C.S], BF16, kind="ExternalInput").ap(),
        "sinT": nc.dram_tensor("sinT", [128, C.S], BF16, kind="ExternalInput").ap(),
        "masks": nc.dram_tensor("masks", [128, 4, 256], F32, kind="ExternalInput").ap(),
        "wnorm": nc.dram_tensor("wnorm", [128, C.HQL + C.KVL], F32,
                                kind="ExternalInput").ap(),
        "out_sh": nc.dram_tensor("out_sh", [C.S, C.COLS], F32,
                                 kind="ExternalOutput").ap(),
    }
    with tile.TileContext(nc) as tc:
        attention_tile_kernel(tc, C, io)
    nc.compile()
    return nc


def make_masks(mask_np, C):
    """4 mask tiles [t,s-pair] for offsets {0,1,WD,WD+1}; returns [128,4,256] f32."""
    S, WD, SW = C.S, C.WD, C.SW
    I0 = WD + 1

    def tileT(d):
        i, j = I0, I0 - d
        if 0 <= j < C.NT:
            blk = np.array(mask_np[i * 128:(i + 1) * 128, j * 128:(j + 1) * 128],
                           dtype=np.float64)
        else:
            blk = np.full((128, 128), -np.inf)
        s_idx = np.arange(128)[:, None]
        t_idx = np.arange(128)[None, :]
        dist = 128 * d + s_idx - t_idx
        blk = np.where(dist > SW, -np.inf, blk)
        return np.maximum(blk.T, -1e30).astype(np.float32)   # [t, s]

    tiles = []
    for off in (0, 1, WD, WD + 1):
        dl, dr = off - 1, off
        tiles.append(np.concatenate([tileT(dl), tileT(dr)], axis=1))
    return np.ascontiguousarray(np.stack(tiles, axis=1))      # [128, 4, 256]


def make_core_inputs(inputs, C):
    x = np.asarray(inputs["x"], dtype=np.float32)
    wq = np.asarray(inputs["wq"], dtype=np.float32)
    wk = np.asarray(inputs["wk"], dtype=np.float32)
    wv = np.asarray(inputs["wv"], dtype=np.float32)
    wo = np.asarray(inputs["wo"], dtype=np.float32)
    qw = np.asarray(inputs["q_norm_weight"], dtype=np.float32)
    kw = np.asarray(inputs["k_norm_weight"], dtype=np.float32)
    ch = np.asarray(inputs["cos_half"], dtype=np.float32)
    sh = np.asarray(inputs["sin_half"], dtype=np.float32)
    mask = np.asarray(inputs["mask"], dtype=np.float32)
    assert int(inputs.get("start_pos", 0) or 0) == 0

    cosT = np.ascontiguousarray(
        np.concatenate([ch.T, ch.T], axis=0)).astype(BF16_NP)
    sinT = np.ascontiguousarray(
        np.concatenate([-sh.T, sh.T], axis=0)).astype(BF16_NP)
    masks = make_masks(mask, C)
    KC, HQL, KVL = C.KC, C.HQL, C.KVL

    xT_cache = {}
    for b in range(C.DP):
        xT_cache[b] = np.ascontiguousarray(x[b].T).astype(BF16_NP).reshape(
            C.KC, 128, C.S)
    in_maps = []
    for c in range(C.NC):
        b, q4 = c // C.TP, c % C.TP
        x16 = xT_cache[b]
        wq_s = wq[:, 128 * HQL * q4:128 * HQL * (q4 + 1)]
        wk_s = wk[:, 128 * KVL * q4:128 * KVL * (q4 + 1)]
        wv_s = wv[:, 128 * KVL * q4:128 * KVL * (q4 + 1)]
        wqk = np.concatenate([wq_s, wk_s], axis=1).astype(BF16_NP)
        # [HCL, 128, KC, 128]: per chain, contraction-partition-major
        wqkv_pre = np.ascontiguousarray(
            wqk.reshape(KC, 128, C.HCL, 128).transpose(2, 1, 0, 3))
        wv_pre = np.ascontiguousarray(
            wv_s.astype(BF16_NP).reshape(KC, 128, KVL * 128).transpose(1, 0, 2))
        wo_s = wo[:, C.COLS * q4:C.COLS * (q4 + 1)].astype(BF16_NP)
        wo_pre = np.ascontiguousarray(
            wo_s.reshape(C.HQ, 128, C.COLS).transpose(1, 0, 2))
        wn = np.zeros((128, HQL + KVL), dtype=np.float32)
        for hc in range(HQL):
            g = HQL * q4 + hc
            wn[:, hc] = qw[128 * g:128 * (g + 1)]
        for j in range(KVL):
            g = KVL * q4 + j
            wn[:, HQL + j] = kw[128 * g:128 * (g + 1)]
        in_maps.append({"xT16": x16, "wqkv": wqkv_pre, "wv_in": wv_pre,
                        "wo_in": wo_pre, "cosT": cosT, "sinT": sinT,
                        "masks": masks, "wnorm": wn})
    return in_maps


_CACHED = {}


def run(inputs, C=None, trace=False, stitch=None, trace_cores=None):
    C = C or Cfg()
    key = (C.S, C.DIM, C.HQ, C.HKV, C.TP, C.DP, C.SW)
    if key not in _CACHED:
        _CACHED[key] = build_program(C)
    nc = _CACHED[key]
    in_maps = make_core_inputs(inputs, C)
    if stitch is None:
        stitch = trace
    if trace and trace_cores is None:
        trace_cores = list(range(C.NC))
    res = bass_utils.run_bass_kernel_spmd(
        nc, in_maps, core_ids=list(range(C.NC)), trace=trace,
        stitch_traces=stitch, trace_cores=trace_cores if trace else None)
    out = np.empty((C.DP, C.S, C.DIM), dtype=np.float32)
    for c in range(C.NC):
        b, q4 = c // C.TP, c % C.TP
        out[b, :, C.COLS * q4:C.COLS * (q4 + 1)] = res.results[c]["out_sh"]
    return out, res


def kernel(**inputs) -> np.ndarray:
    out, _ = run(inputs)
    return out


# revision 27
# speedup vs baseline: 1.0867x; 1.0034x over previous
"""Sparse (sliding-window) GQA attention prefill kernel for 8 Trainium2 cores.

Sharding: TP=4 over KV heads (2 KV heads + 10 Q heads per core) x DP=2 over
batch. Core c: batch = c // 4, shard q4 = c % 4.

Device program (SPMD, identical on all cores; per-core data via inputs):
  A1: xqT/xkT projections in transposed layout ([head_dim, seq]); sum-of-squares
      for the global RMS norm via Square + ones-matmul; per-s-tile AllReduce of
      the norm partials within each batch group (overlaps A1 compute).
  A2: V projection in natural layout ([seq, head_dim]).
  R:  norm rows -> rope tables (bf16, norm factor folded in); batched rope over
      all local heads per 512-slice (vector for q, gpsimd for k), overlapping A2.
  B:  per (head-pair chunk, 2-query-tile group): scoresT = K^T-chunk.T @ qT in
      the sliding band, mask add, exp, ones-matmul denominator, P^T @ V
      accumulation. Software-pipelined: the denominator/PV matmuls for step j
      are emitted after the scores matmul of step j+1 so the PE never waits on
      the exp. Divide on evacuation via broadcast + reciprocal on [128, .].
      attnT shipped in s-halves: AllGather per (chunk, half), issued late so the
      collective's input wait never blocks the gpsimd queue.
  C:  out = attnT.T @ wo col-shard; first s-half rows start as soon as the
      half-0 gathers land.
"""

import sys
import numpy as np

for _p in ("/opt/trn_rl_repo", "/root/.axon_site/_ro/trn_rl_repo"):
    if _p not in sys.path:
        sys.path.insert(0, _p)

import ml_dtypes

import concourse.bass as bass
import concourse.tile as tile
from concourse import bacc, mybir
from concourse import bass_utils

F32 = mybir.dt.float32
BF16 = mybir.dt.bfloat16
BF16_NP = ml_dtypes.bfloat16
AF = mybir.ActivationFunctionType
ALU = mybir.AluOpType


class Cfg:
    def __init__(self, S=2048, DIM=5120, HQ=40, HKV=8, TP=4, DP=2, SW=1024,
                 MSCALE=1.2079441541679836, EPS=1e-6):
        self.S, self.DIM, self.HQ, self.HKV = S, DIM, HQ, HKV
        self.TP, self.DP, self.SW = TP, DP, SW
        self.MSCALE, self.EPS = MSCALE, EPS
        self.D = 128
        self.NC = TP * DP
        self.HQL = HQ // TP          # local q heads
        self.KVL = HKV // TP         # local kv heads
        self.REP = HQ // HKV
        self.KC = DIM // 128         # contraction chunks
        self.NT = S // 128           # seq tiles
        self.G = self.NT // 2        # 2-query-tile groups
        self.WD = SW // 128          # window in tiles
        self.COLS = DIM // TP        # output column shard
        self.HCL = self.HQL + self.KVL  # projection chains with transposed out
        self.NST = S // 512          # 512-wide s-tiles (phase A1)
        self.NST2 = S // 256         # 256-wide s-tiles (phase A2)
        self.CQ = self.D ** -0.5 * MSCALE
        assert self.WD >= 2 and self.NT > self.WD + 1 and self.NT % 2 == 0
        self.groups = [[b * TP + r for r in range(TP)] for b in range(DP)]


def head_chunks(C):
    """Per-kv head pair chunks: [(kv, [h0,h1]), (kv, [h2,h3]), (kv, [h4])...]"""
    out = []
    per = C.HQL // C.KVL
    for kv in range(C.KVL):
        hs = list(range(kv * per, (kv + 1) * per))
        i = 0
        while i < len(hs):
            out.append((kv, hs[i:i + 2]))
            i += 2
    return out


def attention_tile_kernel(tc, C, io):
    nc = tc.nc
    S, KC, HQL, KVL, NT, G, WD = C.S, C.KC, C.HQL, C.KVL, C.NT, C.G, C.WD
    H2 = S // 2
    xT16, wqkv, wv_in, wo_in = io["xT16"], io["wqkv"], io["wv_in"], io["wo_in"]
    cosT_in, sinT_in, masks_in, wnorm_in = io["cosT"], io["sinT"], io["masks"], io["wnorm"]
    out_sh = io["out_sh"]
    chunks = head_chunks(C)

    from contextlib import ExitStack
    ctx = ExitStack()
    with ctx:
        singles = ctx.enter_context(tc.tile_pool(name="singles", bufs=1))
        dramcc = ctx.enter_context(tc.tile_pool(name="dramcc", bufs=1, space="DRAM"))

        ones16 = singles.tile([128, 1], BF16)
        nc.vector.memset(ones16[:], 1.0)
        wnorm_sb = singles.tile([128, HQL + KVL], F32)
        nc.sync.dma_start(wnorm_sb[:], wnorm_in[:])
        cq2 = C.CQ * C.CQ
        eps_q = singles.tile([128, 1], F32)
        nc.vector.memset(eps_q[:], C.EPS / cq2)
        eps_k = singles.tile([128, 1], F32)
        nc.vector.memset(eps_k[:], C.EPS)

        cc_nins = [dramcc.tile([1, 1024], F32, name=f"ccni{st}")
                   for st in range(C.NST)]
        cc_nouts = [dramcc.tile([1, 1024], F32, name=f"ccno{st}")
                    for st in range(C.NST)]
        NCH = len(chunks)
        # full-chunk gathers except the last chunk, which ships in s-halves so
        # phase C's first rows never wait on the final collective
        cc_ains = []
        cc_aouts = []
        for ci, (kv, hs) in enumerate(chunks):
            if ci < NCH - 1:
                cc_ains.append([dramcc.tile([len(hs), 128, S], BF16,
                                            name=f"ccai{ci}")])
                cc_aouts.append([dramcc.tile([C.TP, len(hs), 128, S], BF16,
                                             name=f"ccao{ci}")])
            else:
                cc_ains.append([dramcc.tile([len(hs), 128, H2], BF16,
                                            name=f"ccai{ci}_{hf}")
                                for hf in range(2)])
                cc_aouts.append([dramcc.tile([C.TP, len(hs), 128, H2], BF16,
                                             name=f"ccao{ci}_{hf}")
                                 for hf in range(2)])

        with (
            tc.tile_pool(name="xqp", bufs=1) as xq_pool,
            tc.tile_pool(name="xkp", bufs=1) as xk_pool,
            tc.tile_pool(name="vp", bufs=1) as v_pool,
        ):
            xq_sb = xq_pool.tile([128, HQL, S], BF16)
            xk_sb = xk_pool.tile([128, KVL, S], BF16)

            # ---- phase A1: q/k projections (transposed out) + norm partials --
            KCH = KC // 2
            with (
                tc.tile_pool(name="xt1", bufs=2) as xt1,
                tc.tile_pool(name="wst", bufs=3) as wst,
                tc.tile_pool(name="sqp", bufs=2) as sqp,
                tc.tile_pool(name="trow", bufs=2) as trow,
                tc.tile_pool(name="psA", bufs=3, space="PSUM") as psA,
                tc.tile_pool(name="psN", bufs=1, space="PSUM") as psN,
            ):
                for st in range(C.NST):
                    s0 = st * 512
                    xt_a = xt1.tile([128, KCH, 512], BF16, tag="xta")
                    xt_b = xt1.tile([128, KCH, 512], BF16, tag="xtb")
                    nc.sync.dma_start(
                        xt_a[:],
                        xT16[:KCH, :, s0:s0 + 512].rearrange("kc p s -> p kc s"))
                    nc.sync.dma_start(
                        xt_b[:],
                        xT16[KCH:, :, s0:s0 + 512].rearrange("kc p s -> p kc s"))
                    ps_nq = psN.tile([1, 512], F32, tag="nq")
                    ps_nk = psN.tile([1, 512], F32, tag="nk")
                    for hc in range(C.HCL):
                        w_sb = wst.tile([128, KC, 128], BF16, tag="w")
                        nc.sync.dma_start(w_sb[:], wqkv[hc])
                        ps = psA.tile([128, 512], F32, tag="proj")
                        for kc in range(KC):
                            xsrc = xt_a if kc < KCH else xt_b
                            nc.tensor.matmul(ps[:], w_sb[:, kc, :],
                                             xsrc[:, kc % KCH, :],
                                             start=(kc == 0), stop=(kc == KC - 1))
                        if hc < HQL:
                            dest = xq_sb[:, hc, s0:s0 + 512]
                        else:
                            dest = xk_sb[:, hc - HQL, s0:s0 + 512]
                        nc.vector.tensor_scalar_mul(dest, ps[:],
                                                    wnorm_sb[:, hc:hc + 1])
                        sq = sqp.tile([128, 512], BF16, tag="sq")
                        nc.scalar.activation(sq[:], ps[:], AF.Square)
                        tgt = ps_nq if hc < HQL else ps_nk
                        first = (hc == 0) or (hc == HQL)
                        last = (hc == HQL - 1) or (hc == C.HCL - 1)
                        nc.tensor.matmul(tgt[:], ones16[:], sq[:],
                                         start=first, stop=last)
                    rq_t = trow.tile([1, 512], F32, tag="rq")
                    rk_t = trow.tile([1, 512], F32, tag="rk")
                    nc.vector.tensor_copy(rq_t[:], ps_nq[:])
                    nc.vector.tensor_copy(rk_t[:], ps_nk[:])
                    nc.sync.dma_start(cc_nins[st][0:1, 0:512], rq_t[:])
                    nc.sync.dma_start(cc_nins[st][0:1, 512:1024], rk_t[:])
                    # per-s-tile AllReduce of norm partials: overlaps A1 compute
                    nc.gpsimd.collective_compute(
                        "AllReduce", ALU.add, replica_groups=C.groups,
                        ins=[cc_nins[st].opt()], outs=[cc_nouts[st].opt()])

            # ---- phases R + A2, interleaved emission -------------------------
            # A2's first DMAs are prefetched, then R (norm rows + rope tables +
            # batched rope on vector/gpsimd/scalar) is emitted so it overlaps
            # A2's PE matmuls. A2 evacuates through the scalar engine so R owns
            # the vector queue. R keys off the per-st AllReduces from A1.
            v_sb = v_pool.tile([128, NT, KVL, 128], BF16)
            with (
                tc.tile_pool(name="xt2", bufs=3) as xt2,
                tc.tile_pool(name="wvp", bufs=1) as wvp,
                tc.tile_pool(name="psV", bufs=3, space="PSUM") as psV,
                tc.tile_pool(name="tabsQ", bufs=1) as tabs_q,
                tc.tile_pool(name="rowsp", bufs=1) as rowsp,
                tc.tile_pool(name="ropep", bufs=2) as ropep,
            ):
                wv_sb = wvp.tile([128, KC, KVL * 128], BF16)
                nc.sync.dma_start(wv_sb[:], wv_in[:])
                xts = {}
                for st in range(3):
                    s0 = st * 256
                    xts[st] = xt2.tile([128, KC, 256], BF16, tag="xt2",
                                       name=f"xtp{st}")
                    nc.sync.dma_start(
                        xts[st][:],
                        xT16[:, :, s0:s0 + 256].rearrange("kc p s -> p kc s"))

                # -- phase R emission (per 512-slice, keyed on that slice's
                #    AllReduce so early slices rope during A1's tail) --
                cosq = tabs_q.tile([128, S], BF16)
                sinq = tabs_q.tile([128, S], BF16)
                cosk = tabs_q.tile([128, S], BF16)
                sink = tabs_q.tile([128, S], BF16)
                nc.sync.dma_start(cosk[:], cosT_in[:])
                nc.sync.dma_start(sink[:], sinT_in[:])
                nc.sync.dma_start(cosq[:], cosT_in[:])
                nc.sync.dma_start(sinq[:], sinT_in[:])

                def emit_rchain(st):
                    sl = slice(st * 512, (st + 1) * 512)
                    # broadcast raw partial sums, then r = exp(-0.5*ln(ax+b))
                    rowraw = rowsp.tile([1, 1024], F32, tag="rowraw", bufs=2)
                    nc.sync.dma_start(rowraw[:], cc_nouts[st][:])
                    rq_b = rowsp.tile([128, 512], F32, tag="rqb", bufs=2)
                    rk_b = rowsp.tile([128, 512], F32, tag="rkb", bufs=2)
                    nc.gpsimd.partition_broadcast(rk_b[:],
                                                  rowraw[0:1, 512:1024])
                    nc.gpsimd.partition_broadcast(rq_b[:],
                                                  rowraw[0:1, 0:512])
                    nc.scalar.activation(rk_b[:], rk_b[:], AF.Ln,
                                         scale=1.0 / (C.HKV * 128),
                                         bias=eps_k[:])
                    nc.scalar.activation(rk_b[:], rk_b[:], AF.Exp, scale=-0.5)
                    nc.scalar.activation(rq_b[:], rq_b[:], AF.Ln,
                                         scale=1.0 / (C.DIM * cq2),
                                         bias=eps_q[:])
                    nc.scalar.activation(rq_b[:], rq_b[:], AF.Exp, scale=-0.5)
                    nc.vector.tensor_mul(cosk[:, sl], cosk[:, sl], rk_b[:])
                    nc.vector.tensor_mul(sink[:, sl], sink[:, sl], rk_b[:])
                    nc.vector.tensor_mul(cosq[:, sl], cosq[:, sl], rq_b[:])
                    nc.vector.tensor_mul(sinq[:, sl], sinq[:, sl], rq_b[:])
                    rotk = ropep.tile([128, KVL, 512], BF16, tag="rotk",
                                      bufs=2)
                    rotq = ropep.tile([128, HQL, 512], BF16, tag="rotq")
                    nc.sync.dma_start(rotk[0:64], xk_sb[64:128, :, sl])
                    nc.sync.dma_start(rotk[64:128], xk_sb[0:64, :, sl])
                    nc.sync.dma_start(rotq[0:64], xq_sb[64:128, :, sl])
                    nc.sync.dma_start(rotq[64:128], xq_sb[0:64, :, sl])
                    # fully in-place on vector: rot *= sin; x *= cos; x += rot
                    nc.vector.tensor_mul(
                        rotk[:], rotk[:],
                        sink[:, None, sl].to_broadcast((128, KVL, 512)))
                    nc.vector.tensor_mul(
                        xk_sb[:, :, sl], xk_sb[:, :, sl],
                        cosk[:, None, sl].to_broadcast((128, KVL, 512)))
                    nc.vector.tensor_add(xk_sb[:, :, sl], xk_sb[:, :, sl],
                                         rotk[:])
                    nc.vector.tensor_mul(
                        rotq[:], rotq[:],
                        sinq[:, None, sl].to_broadcast((128, HQL, 512)))
                    nc.vector.tensor_mul(
                        xq_sb[:, :, sl], xq_sb[:, :, sl],
                        cosq[:, None, sl].to_broadcast((128, HQL, 512)))
                    nc.vector.tensor_add(xq_sb[:, :, sl], xq_sb[:, :, sl],
                                         rotq[:])

                for st in range(C.NST - 1):
                    emit_rchain(st)

                # -- phase A2 matmul loop --
                for st in range(C.NST2):
                    s0 = st * 256
                    if st in xts:
                        xt = xts.pop(st)
                    else:
                        xt = xt2.tile([128, KC, 256], BF16, tag="xt2")
                        nc.sync.dma_start(
                            xt[:],
                            xT16[:, :, s0:s0 + 256].rearrange("kc p s -> p kc s"))
                    for tc4 in range(2):
                        tt = 2 * st + tc4
                        psv = psV.tile([128, KVL * 128], F32, tag="v")
                        for kc in range(KC):
                            nc.tensor.matmul(
                                psv[:], xt[:, kc, tc4 * 128:(tc4 + 1) * 128],
                                wv_sb[:, kc, :],
                                start=(kc == 0), stop=(kc == KC - 1))
                        nc.scalar.copy(v_sb[:, tt, :, :], psv[:])

                # last slice's chain after A2: its AllReduce lands ~40us after
                # A1 ends, so emitting it here keeps the scalar/vector queues
                # from blocking A2's evacuations on it.
                emit_rchain(C.NST - 1)

            # ---- phase B: banded attention, software-pipelined --------------
            with (
                tc.tile_pool(name="attnp", bufs=1) as attnp,
                tc.tile_pool(name="maskp", bufs=1) as maskp,
                tc.tile_pool(name="expp", bufs=4) as expp,
                tc.tile_pool(name="bmisc", bufs=3) as bmisc,
                tc.tile_pool(name="psSC", bufs=2, space="PSUM") as psSC,
                tc.tile_pool(name="psAT", bufs=2, space="PSUM") as psAT,
                tc.tile_pool(name="psDN", bufs=2, space="PSUM") as psDN,
            ):
                attnT = attnp.tile([128, HQL, S], BF16)
                masks_sb = maskp.tile([128, 4, 256], F32)
                nc.sync.dma_start(masks_sb[:], masks_in[:])
                off2m = {0: 0, 1: 1, WD: 2, WD + 1: 3}
                # gathers whose issue we delay so the collective's input wait
                # doesn't stall the gpsimd queue ahead of broadcasts
                pend_cc = []

                def flush_cc():
                    while pend_cc:
                        ci_, hf_ = pend_cc.pop(0)
                        nc.gpsimd.collective_compute(
                            "AllGather", ALU.bypass,
                            replica_groups=C.groups,
                            ins=[cc_ains[ci_][hf_].opt()],
                            outs=[cc_aouts[ci_][hf_].opt()])

                for ci, (kv, hs) in enumerate(chunks):
                    w = len(hs)
                    h0 = hs[0]
                    for g in range(G):
                        jlo, jhi = max(0, 2 * g - WD), 2 * g + 1
                        npairs = (jhi - jlo + 1) // 2
                        ps_at = psAT.tile([128, 512], F32, tag="at")
                        ps_dn = psDN.tile([1, 512], F32, tag="dn")
                        pend = []  # (ex slice, j) waiting for dn/at emission

                        def drain():
                            for exp_, jp in pend:
                                nc.tensor.matmul(ps_dn[:, :w * 256], ones16[:],
                                                 exp_, start=(jp == jlo),
                                                 stop=(jp == jhi))
                                nc.tensor.matmul(ps_at[:, :w * 256],
                                                 v_sb[:, jp, kv, :],
                                                 exp_, start=(jp == jlo),
                                                 stop=(jp == jhi))
                            pend.clear()

                        for p in range(npairs):
                            j0 = jlo + 2 * p
                            ps2 = psSC.tile([128, 1024], F32, tag="sc")
                            for dj in range(2):
                                j = j0 + dj
                                o = dj * 512
                                nc.tensor.matmul(
                                    ps2[:, o:o + w * 256].rearrange(
                                        "p (w s) -> p w s", w=w),
                                    xk_sb[:, kv, j * 128:(j + 1) * 128],
                                    xq_sb[:, h0:h0 + w, g * 256:(g + 1) * 256],
                                    start=True, stop=True)
                            # consume the previous pair while this pair's exp
                            # runs -> the PE never waits on the scalar engine
                            drain()
                            for dj in range(2):
                                j = j0 + dj
                                m = off2m.get(jhi - j)
                                if m is not None:
                                    o = dj * 512
                                    nc.vector.tensor_add(
                                        ps2[:, o:o + w * 256].rearrange(
                                            "p (w s) -> p w s", w=w),
                                        ps2[:, o:o + w * 256].rearrange(
                                            "p (w s) -> p w s", w=w),
                                        masks_sb[:, m, None, :].to_broadcast(
                                            (128, w, 256)))
                            ex2 = expp.tile([128, 1024], BF16, tag="ex")
                            nc.scalar.activation(
                                ex2.rearrange("p (j s) -> p j s",
                                              j=2)[:, :, :w * 256],
                                ps2.rearrange("p (j s) -> p j s",
                                              j=2)[:, :, :w * 256],
                                AF.Exp)
                            pend.append((ex2[:, 0:w * 256], j0))
                            pend.append((ex2[:, 512:512 + w * 256], j0 + 1))
                        drain()
                        dn_sb = bmisc.tile([1, 512], F32, tag="dnsb")
                        nc.vector.tensor_copy(dn_sb[:, :w * 256],
                                              ps_dn[:, :w * 256])
                        den_b = bmisc.tile([128, 512], F32, tag="denb")
                        nc.gpsimd.partition_broadcast(den_b[:, :w * 256],
                                                      dn_sb[:, :w * 256])
                        nc.vector.reciprocal_approx_fast(
                            out=den_b[:, :w * 256], in_=den_b[:, :w * 256])
                        nc.vector.tensor_mul(
                            attnT[:, h0:h0 + w, g * 256:(g + 1) * 256],
                            ps_at[:, :w * 256].rearrange(
                                "p (w s) -> p w s", w=w),
                            den_b[:, :w * 256].rearrange(
                                "p (w s) -> p w s", w=w))
                        if ci < NCH - 1:
                            if g == G - 1:
                                nc.sync.dma_start(
                                    cc_ains[ci][0].rearrange("h p s -> p h s"),
                                    attnT[:, h0:h0 + w, :])
                                pend_cc.append((ci, 0))
                        else:
                            if g == G // 2 - 1 or g == G - 1:
                                hf = 0 if g == G // 2 - 1 else 1
                                nc.sync.dma_start(
                                    cc_ains[ci][hf].rearrange("h p s -> p h s"),
                                    attnT[:, h0:h0 + w,
                                          hf * H2:(hf + 1) * H2])
                                pend_cc.append((ci, hf))
                        if g == 1 or (ci == NCH - 1 and g == G // 2 + 1):
                            flush_cc()
                flush_cc()

        # ---- phase C: output projection from gathered attnT ------------------
        with (
            tc.tile_pool(name="wop", bufs=1) as wop,
            tc.tile_pool(name="lhsp", bufs=2) as lhsp,
            tc.tile_pool(name="outp", bufs=2) as outp,
            tc.tile_pool(name="psO", bufs=3, space="PSUM") as psO,
        ):
            wo_sb = wop.tile([128, C.HQ, C.COLS], BF16)
            nc.sync.dma_start(wo_sb[:], wo_in[:])
            # slot -> global head mapping, chunk-major (gather completion order)
            slot_map = []
            base = 0
            chunk_base = []
            for ci, (kv, hs) in enumerate(chunks):
                chunk_base.append(base)
                for r in range(C.TP):
                    for hl in hs:
                        slot_map.append(r * HQL + hl)
                base += C.TP * len(hs)
            col_ts = []
            c0 = 0
            while c0 < C.COLS:
                wdt = min(512, C.COLS - c0)
                col_ts.append((c0, wdt))
                c0 += wdt
            for sb in range(NT):
                hf, sbr = sb // (NT // 2), sb % (NT // 2)
                lhs = lhsp.tile([128, C.HQ, 128], BF16, tag="lhs")
                for ci, (kv, hs) in enumerate(chunks):
                    nslots = C.TP * len(hs)
                    if ci < NCH - 1:
                        src = cc_aouts[ci][0][:, :, :,
                                              sb * 128:(sb + 1) * 128]
                    else:
                        src = cc_aouts[ci][hf][:, :, :,
                                               sbr * 128:(sbr + 1) * 128]
                    nc.sync.dma_start(
                        lhs[:, chunk_base[ci]:chunk_base[ci] + nslots, :],
                        src.rearrange("r h p s -> p (r h) s"))
                orow = outp.tile([128, C.COLS], F32, tag="orow")
                for (c0, wdt) in col_ts:
                    ps_o = psO.tile([128, 512], F32, tag="o")
                    for slot in range(C.HQ):
                        gh = slot_map[slot]
                        nc.tensor.matmul(ps_o[:, :wdt], lhs[:, slot, :],
                                         wo_sb[:, gh, c0:c0 + wdt],
                                         start=(slot == 0), stop=(slot == C.HQ - 1))
                    nc.vector.tensor_copy(orow[:, c0:c0 + wdt], ps_o[:, :wdt])
                nc.sync.dma_start(out_sh[sb * 128:(sb + 1) * 128, :], orow[:])


def build_program(C):
    nc = bacc.Bacc("TRN2", target_bir_lowering=False, debug=False,
                   num_devices=C.NC)
    io = {
        "xT16": nc.dram_tensor("xT16", [C.KC, 128, C.S], BF16, kind="ExternalInput").ap(),
        "wqkv": nc.dram_tensor("wqkv", [C.HCL, 128, C.KC, 128], BF16,
                               kind="ExternalInput").ap(),
        "wv_in": nc.dram_tensor("wv_in", [128, C.KC, C.KVL * 128], BF16,
                                kind="ExternalInput").ap(),
        "wo_in": nc.dram_tensor("wo_in", [128, C.HQ, C.COLS], BF16,
                                kind="ExternalInput").ap(),
        "cosT": nc.dram_tensor("cosT", [128, C.S], BF16, kind="ExternalInput").ap(),
        "sinT": nc.dram_tensor("sinT", [128, C.S], BF16, kind="ExternalInput").ap(),
        "masks": nc.dram_tensor("masks", [128, 4, 256], F32, kind="ExternalInput").ap(),
        "wnorm": nc.dram_tensor("wnorm", [128, C.HQL + C.KVL], F32,
                                kind="ExternalInput").ap(),
        "out_sh": nc.dram_tensor("out_sh", [C.S, C.COLS], F32,
                                 kind="ExternalOutput").ap(),
    }
    with tile.TileContext(nc) as tc:
        attention_tile_kernel(tc, C, io)
    nc.compile()
    return nc


def make_masks(mask_np, C):
    """4 mask tiles [t,s-pair] for offsets {0,1,WD,WD+1}; returns [128,4,256] f32."""
    S, WD, SW = C.S, C.WD, C.SW
    I0 = WD + 1

    def tileT(d):
        i, j = I0, I0 - d
        if 0 <= j < C.NT:
            blk = np.array(mask_np[i * 128:(i + 1) * 128, j * 128:(j + 1) * 128],
                           dtype=np.float64)
        else:
            blk = np.full((128, 128), -np.inf)
        s_idx = np.arange(128)[:, None]
        t_idx = np.arange(128)[None, :]
        dist = 128 * d + s_idx - t_idx
        blk = np.where(dist > SW, -np.inf, blk)
        return np.maximum(blk.T, -1e30).astype(np.float32)   # [t, s]

    tiles = []
    for off in (0, 1, WD, WD + 1):
        dl, dr = off - 1, off
        tiles.append(np.concatenate([tileT(dl), tileT(dr)], axis=1))
    return np.ascontiguousarray(np.stack(tiles, axis=1))      # [128, 4, 256]


def make_core_inputs(inputs, C):
    x = np.asarray(inputs["x"], dtype=np.float32)
    wq = np.asarray(inputs["wq"], dtype=np.float32)
    wk = np.asarray(inputs["wk"], dtype=np.float32)
    wv = np.asarray(inputs["wv"], dtype=np.float32)
    wo = np.asarray(inputs["wo"], dtype=np.float32)
    qw = np.asarray(inputs["q_norm_weight"], dtype=np.float32)
    kw = np.asarray(inputs["k_norm_weight"], dtype=np.float32)
    ch = np.asarray(inputs["cos_half"], dtype=np.float32)
    sh = np.asarray(inputs["sin_half"], dtype=np.float32)
    mask = np.asarray(inputs["mask"], dtype=np.float32)
    assert int(inputs.get("start_pos", 0) or 0) == 0

    cosT = np.ascontiguousarray(
        np.concatenate([ch.T, ch.T], axis=0)).astype(BF16_NP)
    sinT = np.ascontiguousarray(
        np.concatenate([-sh.T, sh.T], axis=0)).astype(BF16_NP)
    masks = make_masks(mask, C)
    KC, HQL, KVL = C.KC, C.HQL, C.KVL

    xT_cache = {}
    for b in range(C.DP):
        xT_cache[b] = np.ascontiguousarray(x[b].T).astype(BF16_NP).reshape(
            C.KC, 128, C.S)
    in_maps = []
    for c in range(C.NC):
        b, q4 = c // C.TP, c % C.TP
        x16 = xT_cache[b]
        wq_s = wq[:, 128 * HQL * q4:128 * HQL * (q4 + 1)]
        wk_s = wk[:, 128 * KVL * q4:128 * KVL * (q4 + 1)]
        wv_s = wv[:, 128 * KVL * q4:128 * KVL * (q4 + 1)]
        wqk = np.concatenate([wq_s, wk_s], axis=1).astype(BF16_NP)
        # [HCL, 128, KC, 128]: per chain, contraction-partition-major
        wqkv_pre = np.ascontiguousarray(
            wqk.reshape(KC, 128, C.HCL, 128).transpose(2, 1, 0, 3))
        wv_pre = np.ascontiguousarray(
            wv_s.astype(BF16_NP).reshape(KC, 128, KVL * 128).transpose(1, 0, 2))
        wo_s = wo[:, C.COLS * q4:C.COLS * (q4 + 1)].astype(BF16_NP)
        wo_pre = np.ascontiguousarray(
            wo_s.reshape(C.HQ, 128, C.COLS).transpose(1, 0, 2))
        wn = np.zeros((128, HQL + KVL), dtype=np.float32)
        for hc in range(HQL):
            g = HQL * q4 + hc
            wn[:, hc] = qw[128 * g:128 * (g + 1)]
        for j in range(KVL):
            g = KVL * q4 + j
            wn[:, HQL + j] = kw[128 * g:128 * (g + 1)]
        in_maps.append({"xT16": x16, "wqkv": wqkv_pre, "wv_in": wv_pre,
                        "wo_in": wo_pre, "cosT": cosT, "sinT": sinT,
                        "masks": masks, "wnorm": wn})
    return in_maps


_CACHED = {}


def run(inputs, C=None, trace=False, stitch=None, trace_cores=None):
    C = C or Cfg()
    key = (C.S, C.DIM, C.HQ, C.HKV, C.TP, C.DP, C.SW)
    if key not in _CACHED:
        _CACHED[key] = build_program(C)
    nc = _CACHED[key]
    in_maps = make_core_inputs(inputs, C)
    if stitch is None:
        stitch = trace
    if trace and trace_cores is None:
        trace_cores = list(range(C.NC))
    res = bass_utils.run_bass_kernel_spmd(
        nc, in_maps, core_ids=list(range(C.NC)), trace=trace,
        stitch_traces=stitch, trace_cores=trace_cores if trace else None)
    out = np.empty((C.DP, C.S, C.DIM), dtype=np.float32)
    for c in range(C.NC):
        b, q4 = c // C.TP, c % C.TP
        out[b, :, C.COLS * q4:C.COLS * (q4 + 1)] = res.results[c]["out_sh"]
    return out, res


def kernel(**inputs) -> np.ndarray:
    out, _ = run(inputs)
    return out


# revision 30
# speedup vs baseline: 1.1298x; 1.0397x over previous
"""Sparse (sliding-window) GQA attention prefill kernel for 8 Trainium2 cores.

Sharding: TP=4 over KV heads (2 KV heads + 10 Q heads per core) x DP=2 over
batch. Core c: batch = c // 4, shard q4 = c % 4.

Device program (SPMD, identical on all cores; per-core data via inputs):
  A1: xqT/xkT projections in transposed layout ([head_dim, seq]); sum-of-squares
      for the global RMS norm via Square + ones-matmul; per-s-tile AllReduce of
      the norm partials within each batch group (overlaps A1 compute).
  A2: V projection in natural layout ([seq, head_dim]).
  R:  norm rows -> rope tables (bf16, norm factor folded in); batched rope over
      all local heads per 512-slice (vector for q, gpsimd for k), overlapping A2.
  B:  per (head-pair chunk, 2-query-tile group): scoresT = K^T-chunk.T @ qT in
      the sliding band, mask add, exp, ones-matmul denominator, P^T @ V
      accumulation. Software-pipelined: the denominator/PV matmuls for step j
      are emitted after the scores matmul of step j+1 so the PE never waits on
      the exp. Divide on evacuation via broadcast + reciprocal on [128, .].
      attnT shipped in s-halves: AllGather per (chunk, half), issued late so the
      collective's input wait never blocks the gpsimd queue.
  C:  out = attnT.T @ wo col-shard; first s-half rows start as soon as the
      half-0 gathers land.
"""

import sys
import numpy as np

for _p in ("/opt/trn_rl_repo", "/root/.axon_site/_ro/trn_rl_repo"):
    if _p not in sys.path:
        sys.path.insert(0, _p)

import ml_dtypes

import concourse.bass as bass
import concourse.tile as tile
from concourse import bacc, mybir
from concourse import bass_utils

F32 = mybir.dt.float32
BF16 = mybir.dt.bfloat16
BF16_NP = ml_dtypes.bfloat16
AF = mybir.ActivationFunctionType
ALU = mybir.AluOpType


class Cfg:
    def __init__(self, S=2048, DIM=5120, HQ=40, HKV=8, TP=4, DP=2, SW=1024,
                 MSCALE=1.2079441541679836, EPS=1e-6):
        self.S, self.DIM, self.HQ, self.HKV = S, DIM, HQ, HKV
        self.TP, self.DP, self.SW = TP, DP, SW
        self.MSCALE, self.EPS = MSCALE, EPS
        self.D = 128
        self.NC = TP * DP
        self.HQL = HQ // TP          # local q heads
        self.KVL = HKV // TP         # local kv heads
        self.REP = HQ // HKV
        self.KC = DIM // 128         # contraction chunks
        self.NT = S // 128           # seq tiles
        self.G = self.NT // 2        # 2-query-tile groups
        self.WD = SW // 128          # window in tiles
        self.COLS = DIM // TP        # output column shard
        self.HCL = self.HQL + self.KVL  # projection chains with transposed out
        self.NST = S // 512          # 512-wide s-tiles (phase A1)
        self.NST2 = S // 256         # 256-wide s-tiles (phase A2)
        self.CQ = self.D ** -0.5 * MSCALE
        assert self.WD >= 2 and self.NT > self.WD + 1 and self.NT % 2 == 0
        self.groups = [[b * TP + r for r in range(TP)] for b in range(DP)]


def head_chunks(C):
    """Per-kv head pair chunks: [(kv, [h0,h1]), (kv, [h2,h3]), (kv, [h4])...]"""
    out = []
    per = C.HQL // C.KVL
    for kv in range(C.KVL):
        hs = list(range(kv * per, (kv + 1) * per))
        i = 0
        while i < len(hs):
            out.append((kv, hs[i:i + 2]))
            i += 2
    return out


def attention_tile_kernel(tc, C, io):
    nc = tc.nc
    S, KC, HQL, KVL, NT, G, WD = C.S, C.KC, C.HQL, C.KVL, C.NT, C.G, C.WD
    H2 = S // 2
    xT16, wqkv, wv_in, wo_in = io["xT16"], io["wqkv"], io["wv_in"], io["wo_in"]
    cosT_in, sinT_in, masks_in, wnorm_in = io["cosT"], io["sinT"], io["masks"], io["wnorm"]
    out_sh = io["out_sh"]
    chunks = head_chunks(C)

    from contextlib import ExitStack
    ctx = ExitStack()
    with ctx:
        singles = ctx.enter_context(tc.tile_pool(name="singles", bufs=1))
        dramcc = ctx.enter_context(tc.tile_pool(name="dramcc", bufs=1, space="DRAM"))

        ones16 = singles.tile([128, 1], BF16)
        nc.vector.memset(ones16[:], 1.0)
        ones128 = singles.tile([128, 128], BF16)
        nc.vector.memset(ones128[:], 1.0)
        wnorm_sb = singles.tile([128, HQL + KVL], F32)
        nc.sync.dma_start(wnorm_sb[:], wnorm_in[:])
        cq2 = C.CQ * C.CQ
        eps_q = singles.tile([128, 1], F32)
        nc.vector.memset(eps_q[:], C.EPS / cq2)
        eps_k = singles.tile([128, 1], F32)
        nc.vector.memset(eps_k[:], C.EPS)

        cc_nins = [dramcc.tile([1, 1024], F32, name=f"ccni{st}")
                   for st in range(C.NST)]
        cc_nouts = [dramcc.tile([1, 1024], F32, name=f"ccno{st}")
                    for st in range(C.NST)]
        NCH = len(chunks)
        # full-chunk gathers except the last chunk, which ships in s-halves so
        # phase C's first rows never wait on the final collective
        cc_ains = []
        cc_aouts = []
        for ci, (kv, hs) in enumerate(chunks):
            if ci < NCH - 1:
                cc_ains.append([dramcc.tile([len(hs), 128, S], BF16,
                                            name=f"ccai{ci}")])
                cc_aouts.append([dramcc.tile([C.TP, len(hs), 128, S], BF16,
                                             name=f"ccao{ci}")])
            else:
                cc_ains.append([dramcc.tile([len(hs), 128, H2], BF16,
                                            name=f"ccai{ci}_{hf}")
                                for hf in range(2)])
                cc_aouts.append([dramcc.tile([C.TP, len(hs), 128, H2], BF16,
                                             name=f"ccao{ci}_{hf}")
                                 for hf in range(2)])

        with (
            tc.tile_pool(name="xqp", bufs=1) as xq_pool,
            tc.tile_pool(name="xkp", bufs=1) as xk_pool,
            tc.tile_pool(name="vp", bufs=1) as v_pool,
        ):
            xq_sb = xq_pool.tile([128, HQL, S], BF16)
            xk_sb = xk_pool.tile([128, KVL, S], BF16)

            # ---- phase A1: q/k projections (transposed out) + norm partials --
            KCH = KC // 2
            with (
                tc.tile_pool(name="xt1", bufs=2) as xt1,
                tc.tile_pool(name="wst", bufs=3) as wst,
                tc.tile_pool(name="sqp", bufs=2) as sqp,
                tc.tile_pool(name="trow", bufs=2) as trow,
                tc.tile_pool(name="psA", bufs=3, space="PSUM") as psA,
                tc.tile_pool(name="psN", bufs=1, space="PSUM") as psN,
            ):
                for st in range(C.NST):
                    s0 = st * 512
                    xt_a = xt1.tile([128, KCH, 512], BF16, tag="xta")
                    xt_b = xt1.tile([128, KCH, 512], BF16, tag="xtb")
                    nc.sync.dma_start(
                        xt_a[:],
                        xT16[:KCH, :, s0:s0 + 512].rearrange("kc p s -> p kc s"))
                    nc.sync.dma_start(
                        xt_b[:],
                        xT16[KCH:, :, s0:s0 + 512].rearrange("kc p s -> p kc s"))
                    ps_nq = psN.tile([1, 512], F32, tag="nq")
                    ps_nk = psN.tile([1, 512], F32, tag="nk")
                    for hc in range(C.HCL):
                        w_sb = wst.tile([128, KC, 128], BF16, tag="w")
                        nc.sync.dma_start(w_sb[:], wqkv[hc])
                        ps = psA.tile([128, 512], F32, tag="proj")
                        for kc in range(KC):
                            xsrc = xt_a if kc < KCH else xt_b
                            nc.tensor.matmul(ps[:], w_sb[:, kc, :],
                                             xsrc[:, kc % KCH, :],
                                             start=(kc == 0), stop=(kc == KC - 1))
                        if hc < HQL:
                            dest = xq_sb[:, hc, s0:s0 + 512]
                        else:
                            dest = xk_sb[:, hc - HQL, s0:s0 + 512]
                        nc.vector.tensor_scalar_mul(dest, ps[:],
                                                    wnorm_sb[:, hc:hc + 1])
                        sq = sqp.tile([128, 512], BF16, tag="sq")
                        nc.scalar.activation(sq[:], ps[:], AF.Square)
                        tgt = ps_nq if hc < HQL else ps_nk
                        first = (hc == 0) or (hc == HQL)
                        last = (hc == HQL - 1) or (hc == C.HCL - 1)
                        nc.tensor.matmul(tgt[:], ones16[:], sq[:],
                                         start=first, stop=last)
                    rq_t = trow.tile([1, 512], F32, tag="rq")
                    rk_t = trow.tile([1, 512], F32, tag="rk")
                    nc.vector.tensor_copy(rq_t[:], ps_nq[:])
                    nc.vector.tensor_copy(rk_t[:], ps_nk[:])
                    nc.sync.dma_start(cc_nins[st][0:1, 0:512], rq_t[:])
                    nc.sync.dma_start(cc_nins[st][0:1, 512:1024], rk_t[:])
                    # per-s-tile AllReduce of norm partials: overlaps A1 compute
                    nc.gpsimd.collective_compute(
                        "AllReduce", ALU.add, replica_groups=C.groups,
                        ins=[cc_nins[st].opt()], outs=[cc_nouts[st].opt()])

            # ---- phases R + A2, interleaved emission -------------------------
            # A2's first DMAs are prefetched, then R (norm rows + rope tables +
            # batched rope on vector/gpsimd/scalar) is emitted so it overlaps
            # A2's PE matmuls. A2 evacuates through the scalar engine so R owns
            # the vector queue. R keys off the per-st AllReduces from A1.
            v_sb = v_pool.tile([128, NT, KVL, 128], BF16)
            with (
                tc.tile_pool(name="xt2", bufs=3) as xt2,
                tc.tile_pool(name="wvp", bufs=1) as wvp,
                tc.tile_pool(name="psV", bufs=3, space="PSUM") as psV,
                tc.tile_pool(name="tabsQ", bufs=1) as tabs_q,
                tc.tile_pool(name="rowsp", bufs=1) as rowsp,
                tc.tile_pool(name="ropep", bufs=2) as ropep,
            ):
                wv_sb = wvp.tile([128, KC, KVL * 128], BF16)
                nc.sync.dma_start(wv_sb[:], wv_in[:])
                xts = {}
                for st in range(3):
                    s0 = st * 256
                    xts[st] = xt2.tile([128, KC, 256], BF16, tag="xt2",
                                       name=f"xtp{st}")
                    nc.sync.dma_start(
                        xts[st][:],
                        xT16[:, :, s0:s0 + 256].rearrange("kc p s -> p kc s"))

                # -- phase R emission (per 512-slice, keyed on that slice's
                #    AllReduce so early slices rope during A1's tail) --
                cosq = tabs_q.tile([128, S], BF16)
                sinq = tabs_q.tile([128, S], BF16)
                cosk = tabs_q.tile([128, S], BF16)
                sink = tabs_q.tile([128, S], BF16)
                nc.sync.dma_start(cosk[:], cosT_in[:])
                nc.sync.dma_start(sink[:], sinT_in[:])
                nc.sync.dma_start(cosq[:], cosT_in[:])
                nc.sync.dma_start(sinq[:], sinT_in[:])

                def emit_rchain(st):
                    sl = slice(st * 512, (st + 1) * 512)
                    # broadcast raw partial sums, then r = exp(-0.5*ln(ax+b))
                    rowraw = rowsp.tile([1, 1024], F32, tag="rowraw", bufs=2)
                    nc.sync.dma_start(rowraw[:], cc_nouts[st][:])
                    rq_b = rowsp.tile([128, 512], F32, tag="rqb", bufs=2)
                    rk_b = rowsp.tile([128, 512], F32, tag="rkb", bufs=2)
                    nc.gpsimd.partition_broadcast(rk_b[:],
                                                  rowraw[0:1, 512:1024])
                    nc.gpsimd.partition_broadcast(rq_b[:],
                                                  rowraw[0:1, 0:512])
                    nc.scalar.activation(rk_b[:], rk_b[:], AF.Ln,
                                         scale=1.0 / (C.HKV * 128),
                                         bias=eps_k[:])
                    nc.scalar.activation(rk_b[:], rk_b[:], AF.Exp, scale=-0.5)
                    nc.scalar.activation(rq_b[:], rq_b[:], AF.Ln,
                                         scale=1.0 / (C.DIM * cq2),
                                         bias=eps_q[:])
                    nc.scalar.activation(rq_b[:], rq_b[:], AF.Exp, scale=-0.5)
                    nc.vector.tensor_mul(cosk[:, sl], cosk[:, sl], rk_b[:])
                    nc.vector.tensor_mul(sink[:, sl], sink[:, sl], rk_b[:])
                    nc.vector.tensor_mul(cosq[:, sl], cosq[:, sl], rq_b[:])
                    nc.vector.tensor_mul(sinq[:, sl], sinq[:, sl], rq_b[:])
                    rotk = ropep.tile([128, KVL, 512], BF16, tag="rotk",
                                      bufs=2)
                    rotq = ropep.tile([128, HQL, 512], BF16, tag="rotq")
                    nc.sync.dma_start(rotk[0:64], xk_sb[64:128, :, sl])
                    nc.sync.dma_start(rotk[64:128], xk_sb[0:64, :, sl])
                    nc.sync.dma_start(rotq[0:64], xq_sb[64:128, :, sl])
                    nc.sync.dma_start(rotq[64:128], xq_sb[0:64, :, sl])
                    # fully in-place on vector: rot *= sin; x *= cos; x += rot
                    nc.vector.tensor_mul(
                        rotk[:], rotk[:],
                        sink[:, None, sl].to_broadcast((128, KVL, 512)))
                    nc.vector.tensor_mul(
                        xk_sb[:, :, sl], xk_sb[:, :, sl],
                        cosk[:, None, sl].to_broadcast((128, KVL, 512)))
                    nc.vector.tensor_add(xk_sb[:, :, sl], xk_sb[:, :, sl],
                                         rotk[:])
                    nc.vector.tensor_mul(
                        rotq[:], rotq[:],
                        sinq[:, None, sl].to_broadcast((128, HQL, 512)))
                    nc.vector.tensor_mul(
                        xq_sb[:, :, sl], xq_sb[:, :, sl],
                        cosq[:, None, sl].to_broadcast((128, HQL, 512)))
                    nc.vector.tensor_add(xq_sb[:, :, sl], xq_sb[:, :, sl],
                                         rotq[:])

                # -- phase A2 matmul loop (emitted first so its xt DMAs lead
                #    the sync queue; R's vector/scalar work keys off the
                #    already-finished AllReduces and overlaps A2's PE) --
                for st in range(C.NST2):
                    s0 = st * 256
                    if st in xts:
                        xt = xts.pop(st)
                    else:
                        xt = xt2.tile([128, KC, 256], BF16, tag="xt2")
                        nc.sync.dma_start(
                            xt[:],
                            xT16[:, :, s0:s0 + 256].rearrange("kc p s -> p kc s"))
                    for tc4 in range(2):
                        tt = 2 * st + tc4
                        psv = psV.tile([128, KVL * 128], F32, tag="v")
                        for kc in range(KC):
                            nc.tensor.matmul(
                                psv[:], xt[:, kc, tc4 * 128:(tc4 + 1) * 128],
                                wv_sb[:, kc, :],
                                start=(kc == 0), stop=(kc == KC - 1))
                        nc.scalar.copy(v_sb[:, tt, :, :], psv[:])

                for st in range(C.NST):
                    emit_rchain(st)

            # ---- phase B: banded attention, software-pipelined --------------
            with (
                tc.tile_pool(name="attnp", bufs=1) as attnp,
                tc.tile_pool(name="maskp", bufs=1) as maskp,
                tc.tile_pool(name="expp", bufs=4) as expp,
                tc.tile_pool(name="bmisc", bufs=3) as bmisc,
                tc.tile_pool(name="psSC", bufs=2, space="PSUM") as psSC,
                tc.tile_pool(name="psAT", bufs=2, space="PSUM") as psAT,
                tc.tile_pool(name="psDN", bufs=2, space="PSUM") as psDN,
            ):
                attnT = attnp.tile([128, HQL, S], BF16)
                masks_sb = maskp.tile([128, 4, 256], F32)
                nc.sync.dma_start(masks_sb[:], masks_in[:])
                off2m = {0: 0, 1: 1, WD: 2, WD + 1: 3}
                # gathers whose issue we delay so the collective's input wait
                # doesn't stall the gpsimd queue ahead of broadcasts
                pend_cc = []

                def flush_cc():
                    while pend_cc:
                        ci_, hf_ = pend_cc.pop(0)
                        nc.gpsimd.collective_compute(
                            "AllGather", ALU.bypass,
                            replica_groups=C.groups,
                            ins=[cc_ains[ci_][hf_].opt()],
                            outs=[cc_aouts[ci_][hf_].opt()])

                for ci, (kv, hs) in enumerate(chunks):
                    w = len(hs)
                    h0 = hs[0]
                    for g in range(G):
                        jlo, jhi = max(0, 2 * g - WD), 2 * g + 1
                        npairs = (jhi - jlo + 1) // 2
                        ps_at = psAT.tile([128, 512], F32, tag="at")
                        # ones128 stationary -> denominator lands pre-broadcast
                        # across all 128 partitions (and avoids the 1-wide
                        # stationary pipeline penalty)
                        ps_dn = psDN.tile([128, 512], F32, tag="dn")
                        pend = []  # (ex slice, j) waiting for dn/at emission

                        def drain():
                            for exp_, jp in pend:
                                nc.tensor.matmul(ps_dn[:, :w * 256],
                                                 ones128[:],
                                                 exp_, start=(jp == jlo),
                                                 stop=(jp == jhi))
                                nc.tensor.matmul(ps_at[:, :w * 256],
                                                 v_sb[:, jp, kv, :],
                                                 exp_, start=(jp == jlo),
                                                 stop=(jp == jhi))
                            pend.clear()

                        for p in range(npairs):
                            j0 = jlo + 2 * p
                            ps2 = psSC.tile([128, 1024], F32, tag="sc")
                            for dj in range(2):
                                j = j0 + dj
                                o = dj * 512
                                nc.tensor.matmul(
                                    ps2[:, o:o + w * 256].rearrange(
                                        "p (w s) -> p w s", w=w),
                                    xk_sb[:, kv, j * 128:(j + 1) * 128],
                                    xq_sb[:, h0:h0 + w, g * 256:(g + 1) * 256],
                                    start=True, stop=True)
                            # consume the previous pair while this pair's exp
                            # runs -> the PE never waits on the scalar engine
                            drain()
                            for dj in range(2):
                                j = j0 + dj
                                m = off2m.get(jhi - j)
                                if m is not None:
                                    o = dj * 512
                                    nc.vector.tensor_add(
                                        ps2[:, o:o + w * 256].rearrange(
                                            "p (w s) -> p w s", w=w),
                                        ps2[:, o:o + w * 256].rearrange(
                                            "p (w s) -> p w s", w=w),
                                        masks_sb[:, m, None, :].to_broadcast(
                                            (128, w, 256)))
                            ex2 = expp.tile([128, 1024], BF16, tag="ex")
                            nc.scalar.activation(
                                ex2.rearrange("p (j s) -> p j s",
                                              j=2)[:, :, :w * 256],
                                ps2.rearrange("p (j s) -> p j s",
                                              j=2)[:, :, :w * 256],
                                AF.Exp)
                            pend.append((ex2[:, 0:w * 256], j0))
                            pend.append((ex2[:, 512:512 + w * 256], j0 + 1))
                        drain()
                        den_b = bmisc.tile([128, 512], F32, tag="denb")
                        nc.vector.reciprocal_approx_fast(
                            out=den_b[:, :w * 256], in_=ps_dn[:, :w * 256])
                        nc.vector.tensor_mul(
                            attnT[:, h0:h0 + w, g * 256:(g + 1) * 256],
                            ps_at[:, :w * 256].rearrange(
                                "p (w s) -> p w s", w=w),
                            den_b[:, :w * 256].rearrange(
                                "p (w s) -> p w s", w=w))
                        if ci < NCH - 1:
                            if g == G - 1:
                                nc.sync.dma_start(
                                    cc_ains[ci][0].rearrange("h p s -> p h s"),
                                    attnT[:, h0:h0 + w, :])
                                pend_cc.append((ci, 0))
                        else:
                            if g == G // 2 - 1 or g == G - 1:
                                hf = 0 if g == G // 2 - 1 else 1
                                nc.sync.dma_start(
                                    cc_ains[ci][hf].rearrange("h p s -> p h s"),
                                    attnT[:, h0:h0 + w,
                                          hf * H2:(hf + 1) * H2])
                                pend_cc.append((ci, hf))
                        if g == 1 or (ci == NCH - 1 and g == G // 2 + 1):
                            flush_cc()
                flush_cc()

        # ---- phase C: output projection from gathered attnT ------------------
        with (
            tc.tile_pool(name="wop", bufs=1) as wop,
            tc.tile_pool(name="lhsp", bufs=2) as lhsp,
            tc.tile_pool(name="outp", bufs=2) as outp,
            tc.tile_pool(name="psO", bufs=3, space="PSUM") as psO,
        ):
            wo_sb = wop.tile([128, C.HQ, C.COLS], BF16)
            nc.sync.dma_start(wo_sb[:], wo_in[:])
            # slot -> global head mapping, chunk-major (gather completion order)
            slot_map = []
            base = 0
            chunk_base = []
            for ci, (kv, hs) in enumerate(chunks):
                chunk_base.append(base)
                for r in range(C.TP):
                    for hl in hs:
                        slot_map.append(r * HQL + hl)
                base += C.TP * len(hs)
            col_ts = []
            c0 = 0
            while c0 < C.COLS:
                wdt = min(512, C.COLS - c0)
                col_ts.append((c0, wdt))
                c0 += wdt
            for sb in range(NT):
                hf, sbr = sb // (NT // 2), sb % (NT // 2)
                lhs = lhsp.tile([128, C.HQ, 128], BF16, tag="lhs")
                for ci, (kv, hs) in enumerate(chunks):
                    nslots = C.TP * len(hs)
                    if ci < NCH - 1:
                        src = cc_aouts[ci][0][:, :, :,
                                              sb * 128:(sb + 1) * 128]
                    else:
                        src = cc_aouts[ci][hf][:, :, :,
                                               sbr * 128:(sbr + 1) * 128]
                    nc.sync.dma_start(
                        lhs[:, chunk_base[ci]:chunk_base[ci] + nslots, :],
                        src.rearrange("r h p s -> p (r h) s"))
                orow = outp.tile([128, C.COLS], F32, tag="orow")
                for (c0, wdt) in col_ts:
                    ps_o = psO.tile([128, 512], F32, tag="o")
                    for slot in range(C.HQ):
                        gh = slot_map[slot]
                        nc.tensor.matmul(ps_o[:, :wdt], lhs[:, slot, :],
                                         wo_sb[:, gh, c0:c0 + wdt],
                                         start=(slot == 0), stop=(slot == C.HQ - 1))
                    nc.vector.tensor_copy(orow[:, c0:c0 + wdt], ps_o[:, :wdt])
                nc.sync.dma_start(out_sh[sb * 128:(sb + 1) * 128, :], orow[:])


def build_program(C):
    nc = bacc.Bacc("TRN2", target_bir_lowering=False, debug=False,
                   num_devices=C.NC)
    io = {
        "xT16": nc.dram_tensor("xT16", [C.KC, 128, C.S], BF16, kind="ExternalInput").ap(),
        "wqkv": nc.dram_tensor("wqkv", [C.HCL, 128, C.KC, 128], BF16,
                               kind="ExternalInput").ap(),
        "wv_in": nc.dram_tensor("wv_in", [128, C.KC, C.KVL * 128], BF16,
                                kind="ExternalInput").ap(),
        "wo_in": nc.dram_tensor("wo_in", [128, C.HQ, C.COLS], BF16,
                                kind="ExternalInput").ap(),
        "cosT": nc.dram_tensor("cosT", [128, C.S], BF16, kind="ExternalInput").ap(),
        "sinT": nc.dram_tensor("sinT", [128, C.S], BF16, kind="ExternalInput").ap(),
        "masks": nc.dram_tensor("masks", [128, 4, 256], F32, kind="ExternalInput").ap(),
        "wnorm": nc.dram_tensor("wnorm", [128, C.HQL + C.KVL], F32,
                                kind="ExternalInput").ap(),
        "out_sh": nc.dram_tensor("out_sh", [C.S, C.COLS], F32,
                                 kind="ExternalOutput").ap(),
    }
    with tile.TileContext(nc) as tc:
        attention_tile_kernel(tc, C, io)
    nc.compile()
    return nc


def make_masks(mask_np, C):
    """4 mask tiles [t,s-pair] for offsets {0,1,WD,WD+1}; returns [128,4,256] f32."""
    S, WD, SW = C.S, C.WD, C.SW
    I0 = WD + 1

    def tileT(d):
        i, j = I0, I0 - d
        if 0 <= j < C.NT:
            blk = np.array(mask_np[i * 128:(i + 1) * 128, j * 128:(j + 1) * 128],
                           dtype=np.float64)
        else:
            blk = np.full((128, 128), -np.inf)
        s_idx = np.arange(128)[:, None]
        t_idx = np.arange(128)[None, :]
        dist = 128 * d + s_idx - t_idx
        blk = np.where(dist > SW, -np.inf, blk)
        return np.maximum(blk.T, -1e30).astype(np.float32)   # [t, s]

    tiles = []
    for off in (0, 1, WD, WD + 1):
        dl, dr = off - 1, off
        tiles.append(np.concatenate([tileT(dl), tileT(dr)], axis=1))
    return np.ascontiguousarray(np.stack(tiles, axis=1))      # [128, 4, 256]


def make_core_inputs(inputs, C):
    x = np.asarray(inputs["x"], dtype=np.float32)
    wq = np.asarray(inputs["wq"], dtype=np.float32)
    wk = np.asarray(inputs["wk"], dtype=np.float32)
    wv = np.asarray(inputs["wv"], dtype=np.float32)
    wo = np.asarray(inputs["wo"], dtype=np.float32)
    qw = np.asarray(inputs["q_norm_weight"], dtype=np.float32)
    kw = np.asarray(inputs["k_norm_weight"], dtype=np.float32)
    ch = np.asarray(inputs["cos_half"], dtype=np.float32)
    sh = np.asarray(inputs["sin_half"], dtype=np.float32)
    mask = np.asarray(inputs["mask"], dtype=np.float32)
    assert int(inputs.get("start_pos", 0) or 0) == 0

    cosT = np.ascontiguousarray(
        np.concatenate([ch.T, ch.T], axis=0)).astype(BF16_NP)
    sinT = np.ascontiguousarray(
        np.concatenate([-sh.T, sh.T], axis=0)).astype(BF16_NP)
    masks = make_masks(mask, C)
    KC, HQL, KVL = C.KC, C.HQL, C.KVL

    xT_cache = {}
    for b in range(C.DP):
        xT_cache[b] = np.ascontiguousarray(x[b].T).astype(BF16_NP).reshape(
            C.KC, 128, C.S)
    in_maps = []
    for c in range(C.NC):
        b, q4 = c // C.TP, c % C.TP
        x16 = xT_cache[b]
        wq_s = wq[:, 128 * HQL * q4:128 * HQL * (q4 + 1)]
        wk_s = wk[:, 128 * KVL * q4:128 * KVL * (q4 + 1)]
        wv_s = wv[:, 128 * KVL * q4:128 * KVL * (q4 + 1)]
        wqk = np.concatenate([wq_s, wk_s], axis=1).astype(BF16_NP)
        # [HCL, 128, KC, 128]: per chain, contraction-partition-major
        wqkv_pre = np.ascontiguousarray(
            wqk.reshape(KC, 128, C.HCL, 128).transpose(2, 1, 0, 3))
        wv_pre = np.ascontiguousarray(
            wv_s.astype(BF16_NP).reshape(KC, 128, KVL * 128).transpose(1, 0, 2))
        wo_s = wo[:, C.COLS * q4:C.COLS * (q4 + 1)].astype(BF16_NP)
        wo_pre = np.ascontiguousarray(
            wo_s.reshape(C.HQ, 128, C.COLS).transpose(1, 0, 2))
        wn = np.zeros((128, HQL + KVL), dtype=np.float32)
        for hc in range(HQL):
            g = HQL * q4 + hc
            wn[:, hc] = qw[128 * g:128 * (g + 1)]
        for j in range(KVL):
            g = KVL * q4 + j
            wn[:, HQL + j] = kw[128 * g:128 * (g + 1)]
        in_maps.append({"xT16": x16, "wqkv": wqkv_pre, "wv_in": wv_pre,
                        "wo_in": wo_pre, "cosT": cosT, "sinT": sinT,
                        "masks": masks, "wnorm": wn})
    return in_maps


_CACHED = {}


def run(inputs, C=None, trace=False, stitch=None, trace_cores=None):
    C = C or Cfg()
    key = (C.S, C.DIM, C.HQ, C.HKV, C.TP, C.DP, C.SW)
    if key not in _CACHED:
        _CACHED[key] = build_program(C)
    nc = _CACHED[key]
    in_maps = make_core_inputs(inputs, C)
    if stitch is None:
        stitch = trace
    if trace and trace_cores is None:
        trace_cores = list(range(C.NC))
    res = bass_utils.run_bass_kernel_spmd(
        nc, in_maps, core_ids=list(range(C.NC)), trace=trace,
        stitch_traces=stitch, trace_cores=trace_cores if trace else None)
    out = np.empty((C.DP, C.S, C.DIM), dtype=np.float32)
    for c in range(C.NC):
        b, q4 = c // C.TP, c % C.TP
        out[b, :, C.COLS * q4:C.COLS * (q4 + 1)] = res.results[c]["out_sh"]
    return out, res


def kernel(**inputs) -> np.ndarray:
    out, _ = run(inputs)
    return out


# revision 34
# speedup vs baseline: 1.1479x; 1.0160x over previous
"""Sparse (sliding-window) GQA attention prefill kernel for 8 Trainium2 cores.

Sharding: TP=4 over KV heads (2 KV heads + 10 Q heads per core) x DP=2 over
batch. Core c: batch = c // 4, shard q4 = c % 4.

Device program (SPMD, identical on all cores; per-core data via inputs):
  A1: xqT/xkT projections in transposed layout ([head_dim, seq]); sum-of-squares
      for the global RMS norm via Square + ones-matmul; per-s-tile AllReduce of
      the norm partials within each batch group (overlaps A1 compute).
  A2: V projection in natural layout ([seq, head_dim]).
  R:  norm rows -> rope tables (bf16, norm factor folded in); batched rope over
      all local heads per 512-slice (vector for q, gpsimd for k), overlapping A2.
  B:  per (head-pair chunk, 2-query-tile group): scoresT = K^T-chunk.T @ qT in
      the sliding band, mask add, exp, ones-matmul denominator, P^T @ V
      accumulation. Software-pipelined: the denominator/PV matmuls for step j
      are emitted after the scores matmul of step j+1 so the PE never waits on
      the exp. Divide on evacuation via broadcast + reciprocal on [128, .].
      attnT shipped in s-halves: AllGather per (chunk, half), issued late so the
      collective's input wait never blocks the gpsimd queue.
  C:  out = attnT.T @ wo col-shard; first s-half rows start as soon as the
      half-0 gathers land.
"""

import sys
import numpy as np

for _p in ("/opt/trn_rl_repo", "/root/.axon_site/_ro/trn_rl_repo"):
    if _p not in sys.path:
        sys.path.insert(0, _p)

import ml_dtypes

import concourse.bass as bass
import concourse.tile as tile
from concourse import bacc, mybir
from concourse import bass_utils

F32 = mybir.dt.float32
BF16 = mybir.dt.bfloat16
BF16_NP = ml_dtypes.bfloat16
AF = mybir.ActivationFunctionType
ALU = mybir.AluOpType


class Cfg:
    def __init__(self, S=2048, DIM=5120, HQ=40, HKV=8, TP=4, DP=2, SW=1024,
                 MSCALE=1.2079441541679836, EPS=1e-6):
        self.S, self.DIM, self.HQ, self.HKV = S, DIM, HQ, HKV
        self.TP, self.DP, self.SW = TP, DP, SW
        self.MSCALE, self.EPS = MSCALE, EPS
        self.D = 128
        self.NC = TP * DP
        self.HQL = HQ // TP          # local q heads
        self.KVL = HKV // TP         # local kv heads
        self.REP = HQ // HKV
        self.KC = DIM // 128         # contraction chunks
        self.NT = S // 128           # seq tiles
        self.G = self.NT // 2        # 2-query-tile groups
        self.WD = SW // 128          # window in tiles
        self.COLS = DIM // TP        # output column shard
        self.HCL = self.HQL + self.KVL  # projection chains with transposed out
        self.NST = S // 512          # 512-wide s-tiles (phase A1)
        self.NST2 = S // 256         # 256-wide s-tiles (phase A2)
        self.CQ = self.D ** -0.5 * MSCALE
        assert self.WD >= 2 and self.NT > self.WD + 1 and self.NT % 2 == 0
        self.groups = [[b * TP + r for r in range(TP)] for b in range(DP)]


def head_chunks(C):
    """Per-kv head pair chunks: [(kv, [h0,h1]), (kv, [h2,h3]), (kv, [h4])...]"""
    out = []
    per = C.HQL // C.KVL
    for kv in range(C.KVL):
        hs = list(range(kv * per, (kv + 1) * per))
        i = 0
        while i < len(hs):
            out.append((kv, hs[i:i + 2]))
            i += 2
    return out


def attention_tile_kernel(tc, C, io):
    nc = tc.nc
    S, KC, HQL, KVL, NT, G, WD = C.S, C.KC, C.HQL, C.KVL, C.NT, C.G, C.WD
    H2 = S // 2
    xT16, wqkv, wv_in, wo_in = io["xT16"], io["wqkv"], io["wv_in"], io["wo_in"]
    cosT_in, sinT_in, masks_in, wnorm_in = io["cosT"], io["sinT"], io["masks"], io["wnorm"]
    out_sh = io["out_sh"]
    chunks = head_chunks(C)

    from contextlib import ExitStack
    ctx = ExitStack()
    with ctx:
        singles = ctx.enter_context(tc.tile_pool(name="singles", bufs=1))
        dramcc = ctx.enter_context(tc.tile_pool(name="dramcc", bufs=1, space="DRAM"))

        ones16 = singles.tile([128, 1], BF16)
        nc.vector.memset(ones16[:], 1.0)
        ones128 = singles.tile([128, 128], BF16)
        nc.vector.memset(ones128[:], 1.0)
        wnorm_sb = singles.tile([128, HQL + KVL], F32)
        nc.sync.dma_start(wnorm_sb[:], wnorm_in[:])
        cq2 = C.CQ * C.CQ
        eps_q = singles.tile([128, 1], F32)
        nc.vector.memset(eps_q[:], C.EPS / cq2)
        eps_k = singles.tile([128, 1], F32)
        nc.vector.memset(eps_k[:], C.EPS)

        cc_nins = [dramcc.tile([1, 1024], F32, name=f"ccni{st}")
                   for st in range(C.NST)]
        cc_nouts = [dramcc.tile([1, 1024], F32, name=f"ccno{st}")
                    for st in range(C.NST)]
        # one gather per 256-wide s-range (phase B runs g-outer), so phase C
        # consumes s-slabs progressively and never waits on the last collective
        cc_ains = [dramcc.tile([HQL, 128, 256], BF16, name=f"ccag{g}")
                   for g in range(G)]
        cc_aouts = [dramcc.tile([C.TP, HQL, 128, 256], BF16,
                                name=f"ccaog{g}") for g in range(G)]

        with (
            tc.tile_pool(name="xqp", bufs=1) as xq_pool,
            tc.tile_pool(name="xkp", bufs=1) as xk_pool,
            tc.tile_pool(name="vp", bufs=1) as v_pool,
        ):
            xq_sb = xq_pool.tile([128, HQL, S], BF16)
            xk_sb = xk_pool.tile([128, KVL, S], BF16)

            # ---- phase A1: q/k projections (transposed out) + norm partials --
            KCH = KC // 2
            with (
                tc.tile_pool(name="xt1", bufs=2) as xt1,
                tc.tile_pool(name="wst", bufs=3) as wst,
                tc.tile_pool(name="sqp", bufs=2) as sqp,
                tc.tile_pool(name="trow", bufs=2) as trow,
                tc.tile_pool(name="psA", bufs=3, space="PSUM") as psA,
                tc.tile_pool(name="psN", bufs=1, space="PSUM") as psN,
            ):
                for st in range(C.NST):
                    s0 = st * 512
                    xt_a = xt1.tile([128, KCH, 512], BF16, tag="xta")
                    xt_b = xt1.tile([128, KCH, 512], BF16, tag="xtb")
                    nc.sync.dma_start(
                        xt_a[:],
                        xT16[:KCH, :, s0:s0 + 512].rearrange("kc p s -> p kc s"))
                    nc.sync.dma_start(
                        xt_b[:],
                        xT16[KCH:, :, s0:s0 + 512].rearrange("kc p s -> p kc s"))
                    ps_nq = psN.tile([1, 512], F32, tag="nq")
                    ps_nk = psN.tile([1, 512], F32, tag="nk")
                    for hc in range(C.HCL):
                        w_sb = wst.tile([128, KC, 128], BF16, tag="w")
                        nc.sync.dma_start(w_sb[:], wqkv[hc])
                        ps = psA.tile([128, 512], F32, tag="proj")
                        for kc in range(KC):
                            xsrc = xt_a if kc < KCH else xt_b
                            nc.tensor.matmul(ps[:], w_sb[:, kc, :],
                                             xsrc[:, kc % KCH, :],
                                             start=(kc == 0), stop=(kc == KC - 1))
                        if hc < HQL:
                            dest = xq_sb[:, hc, s0:s0 + 512]
                        else:
                            dest = xk_sb[:, hc - HQL, s0:s0 + 512]
                        nc.vector.tensor_scalar_mul(dest, ps[:],
                                                    wnorm_sb[:, hc:hc + 1])
                        sq = sqp.tile([128, 512], BF16, tag="sq")
                        nc.scalar.activation(sq[:], ps[:], AF.Square)
                        tgt = ps_nq if hc < HQL else ps_nk
                        first = (hc == 0) or (hc == HQL)
                        last = (hc == HQL - 1) or (hc == C.HCL - 1)
                        nc.tensor.matmul(tgt[:], ones16[:], sq[:],
                                         start=first, stop=last)
                    rq_t = trow.tile([1, 512], F32, tag="rq")
                    rk_t = trow.tile([1, 512], F32, tag="rk")
                    nc.vector.tensor_copy(rq_t[:], ps_nq[:])
                    nc.vector.tensor_copy(rk_t[:], ps_nk[:])
                    nc.sync.dma_start(cc_nins[st][0:1, 0:512], rq_t[:])
                    nc.sync.dma_start(cc_nins[st][0:1, 512:1024], rk_t[:])
                    # per-s-tile AllReduce of norm partials: overlaps A1 compute
                    nc.gpsimd.collective_compute(
                        "AllReduce", ALU.add, replica_groups=C.groups,
                        ins=[cc_nins[st].opt()], outs=[cc_nouts[st].opt()])

            # ---- phases R + A2, interleaved emission -------------------------
            # A2's first DMAs are prefetched, then R (norm rows + rope tables +
            # batched rope on vector/gpsimd/scalar) is emitted so it overlaps
            # A2's PE matmuls. A2 evacuates through the scalar engine so R owns
            # the vector queue. R keys off the per-st AllReduces from A1.
            v_sb = v_pool.tile([128, NT, KVL, 128], BF16)
            with (
                tc.tile_pool(name="xt2", bufs=3) as xt2,
                tc.tile_pool(name="wvp", bufs=1) as wvp,
                tc.tile_pool(name="psV", bufs=3, space="PSUM") as psV,
                tc.tile_pool(name="tabsQ", bufs=1) as tabs_q,
                tc.tile_pool(name="rowsp", bufs=1) as rowsp,
                tc.tile_pool(name="ropep", bufs=2) as ropep,
            ):
                wv_sb = wvp.tile([128, KC, KVL * 128], BF16)
                nc.sync.dma_start(wv_sb[:], wv_in[:])
                xts = {}
                for st in range(3):
                    s0 = st * 256
                    xts[st] = xt2.tile([128, KC, 256], BF16, tag="xt2",
                                       name=f"xtp{st}")
                    nc.sync.dma_start(
                        xts[st][:],
                        xT16[:, :, s0:s0 + 256].rearrange("kc p s -> p kc s"))

                # -- phase R emission (per 512-slice, keyed on that slice's
                #    AllReduce so early slices rope during A1's tail) --
                cosq = tabs_q.tile([128, S], BF16)
                sinq = tabs_q.tile([128, S], BF16)
                cosk = tabs_q.tile([128, S], BF16)
                sink = tabs_q.tile([128, S], BF16)
                nc.gpsimd.dma_start(cosk[:], cosT_in[:])
                nc.gpsimd.dma_start(sink[:], sinT_in[:])
                nc.gpsimd.dma_start(cosq[:], cosT_in[:])
                nc.gpsimd.dma_start(sinq[:], sinT_in[:])

                def emit_rchain(st):
                    sl = slice(st * 512, (st + 1) * 512)
                    # broadcast raw partial sums, then r = exp(-0.5*ln(ax+b))
                    rowraw = rowsp.tile([1, 1024], F32, tag="rowraw", bufs=2)
                    nc.gpsimd.dma_start(rowraw[:], cc_nouts[st][:])
                    rq_b = rowsp.tile([128, 512], F32, tag="rqb", bufs=2)
                    rk_b = rowsp.tile([128, 512], F32, tag="rkb", bufs=2)
                    nc.gpsimd.partition_broadcast(rk_b[:],
                                                  rowraw[0:1, 512:1024])
                    nc.gpsimd.partition_broadcast(rq_b[:],
                                                  rowraw[0:1, 0:512])
                    nc.scalar.activation(rk_b[:], rk_b[:], AF.Ln,
                                         scale=1.0 / (C.HKV * 128),
                                         bias=eps_k[:])
                    nc.scalar.activation(rk_b[:], rk_b[:], AF.Exp, scale=-0.5)
                    nc.scalar.activation(rq_b[:], rq_b[:], AF.Ln,
                                         scale=1.0 / (C.DIM * cq2),
                                         bias=eps_q[:])
                    nc.scalar.activation(rq_b[:], rq_b[:], AF.Exp, scale=-0.5)
                    nc.vector.tensor_mul(cosk[:, sl], cosk[:, sl], rk_b[:])
                    nc.vector.tensor_mul(sink[:, sl], sink[:, sl], rk_b[:])
                    nc.vector.tensor_mul(cosq[:, sl], cosq[:, sl], rq_b[:])
                    nc.vector.tensor_mul(sinq[:, sl], sinq[:, sl], rq_b[:])
                    rotk = ropep.tile([128, KVL, 512], BF16, tag="rotk",
                                      bufs=2)
                    rotq = ropep.tile([128, HQL, 512], BF16, tag="rotq")
                    nc.gpsimd.dma_start(rotk[0:64], xk_sb[64:128, :, sl])
                    nc.gpsimd.dma_start(rotk[64:128], xk_sb[0:64, :, sl])
                    nc.gpsimd.dma_start(rotq[0:64], xq_sb[64:128, :, sl])
                    nc.gpsimd.dma_start(rotq[64:128], xq_sb[0:64, :, sl])
                    # fully in-place on vector: rot *= sin; x *= cos; x += rot
                    nc.vector.tensor_mul(
                        rotk[:], rotk[:],
                        sink[:, None, sl].to_broadcast((128, KVL, 512)))
                    nc.vector.tensor_mul(
                        xk_sb[:, :, sl], xk_sb[:, :, sl],
                        cosk[:, None, sl].to_broadcast((128, KVL, 512)))
                    nc.vector.tensor_add(xk_sb[:, :, sl], xk_sb[:, :, sl],
                                         rotk[:])
                    nc.vector.tensor_mul(
                        rotq[:], rotq[:],
                        sinq[:, None, sl].to_broadcast((128, HQL, 512)))
                    nc.vector.tensor_mul(
                        xq_sb[:, :, sl], xq_sb[:, :, sl],
                        cosq[:, None, sl].to_broadcast((128, HQL, 512)))
                    nc.vector.tensor_add(xq_sb[:, :, sl], xq_sb[:, :, sl],
                                         rotq[:])

                # chains for the first slices: their AllReduces complete
                # during A1, so this vector/scalar/gpsimd work overlaps A2's
                # PE. The last slice's chain is emitted after A2 because its
                # AllReduce lands ~40us after A1 ends.
                for st in range(C.NST - 1):
                    emit_rchain(st)

                for st in range(C.NST2):
                    s0 = st * 256
                    if st in xts:
                        xt = xts.pop(st)
                    else:
                        xt = xt2.tile([128, KC, 256], BF16, tag="xt2")
                        nc.sync.dma_start(
                            xt[:],
                            xT16[:, :, s0:s0 + 256].rearrange("kc p s -> p kc s"))
                    for tc4 in range(2):
                        tt = 2 * st + tc4
                        psv = psV.tile([128, KVL * 128], F32, tag="v")
                        for kc in range(KC):
                            nc.tensor.matmul(
                                psv[:], xt[:, kc, tc4 * 128:(tc4 + 1) * 128],
                                wv_sb[:, kc, :],
                                start=(kc == 0), stop=(kc == KC - 1))
                        nc.scalar.copy(v_sb[:, tt, :, :], psv[:])

                emit_rchain(C.NST - 1)

            # ---- phase B: banded attention, software-pipelined --------------
            with (
                tc.tile_pool(name="attnp", bufs=1) as attnp,
                tc.tile_pool(name="maskp", bufs=1) as maskp,
                tc.tile_pool(name="expp", bufs=4) as expp,
                tc.tile_pool(name="bmisc", bufs=3) as bmisc,
                tc.tile_pool(name="psSC", bufs=2, space="PSUM") as psSC,
                tc.tile_pool(name="psAT", bufs=2, space="PSUM") as psAT,
                tc.tile_pool(name="psDN", bufs=2, space="PSUM") as psDN,
            ):
                attnT = attnp.tile([128, HQL, S], BF16)
                masks_sb = maskp.tile([128, 4, 256], F32)
                nc.sync.dma_start(masks_sb[:], masks_in[:])
                off2m = {0: 0, 1: 1, WD: 2, WD + 1: 3}

                for g in range(G):
                    jlo, jhi = max(0, 2 * g - WD), 2 * g + 1
                    npairs = (jhi - jlo + 1) // 2
                    for ci, (kv, hs) in enumerate(chunks):
                        w = len(hs)
                        h0 = hs[0]
                        ps_at = psAT.tile([128, 512], F32, tag="at")
                        # ones128 stationary -> denominator lands pre-broadcast
                        # across all 128 partitions (and avoids the 1-wide
                        # stationary pipeline penalty)
                        ps_dn = psDN.tile([128, 512], F32, tag="dn")
                        pend = []  # (ex slice, j) waiting for dn/at emission

                        def drain():
                            for exp_, jp in pend:
                                nc.tensor.matmul(ps_dn[:, :w * 256],
                                                 ones128[:],
                                                 exp_, start=(jp == jlo),
                                                 stop=(jp == jhi))
                                nc.tensor.matmul(ps_at[:, :w * 256],
                                                 v_sb[:, jp, kv, :],
                                                 exp_, start=(jp == jlo),
                                                 stop=(jp == jhi))
                            pend.clear()

                        for p in range(npairs):
                            j0 = jlo + 2 * p
                            ps2 = psSC.tile([128, 1024], F32, tag="sc")
                            for dj in range(2):
                                j = j0 + dj
                                o = dj * 512
                                nc.tensor.matmul(
                                    ps2[:, o:o + w * 256].rearrange(
                                        "p (w s) -> p w s", w=w),
                                    xk_sb[:, kv, j * 128:(j + 1) * 128],
                                    xq_sb[:, h0:h0 + w, g * 256:(g + 1) * 256],
                                    start=True, stop=True)
                            # consume the previous pair while this pair's exp
                            # runs -> the PE never waits on the scalar engine
                            drain()
                            for dj in range(2):
                                j = j0 + dj
                                m = off2m.get(jhi - j)
                                if m is not None:
                                    o = dj * 512
                                    nc.vector.tensor_add(
                                        ps2[:, o:o + w * 256].rearrange(
                                            "p (w s) -> p w s", w=w),
                                        ps2[:, o:o + w * 256].rearrange(
                                            "p (w s) -> p w s", w=w),
                                        masks_sb[:, m, None, :].to_broadcast(
                                            (128, w, 256)))
                            ex2 = expp.tile([128, 1024], BF16, tag="ex")
                            nc.scalar.activation(
                                ex2.rearrange("p (j s) -> p j s",
                                              j=2)[:, :, :w * 256],
                                ps2.rearrange("p (j s) -> p j s",
                                              j=2)[:, :, :w * 256],
                                AF.Exp)
                            pend.append((ex2[:, 0:w * 256], j0))
                            pend.append((ex2[:, 512:512 + w * 256], j0 + 1))
                        drain()
                        den_b = bmisc.tile([128, 512], F32, tag="denb")
                        nc.vector.reciprocal_approx_fast(
                            out=den_b[:, :w * 256], in_=ps_dn[:, :w * 256])
                        nc.vector.tensor_mul(
                            attnT[:, h0:h0 + w, g * 256:(g + 1) * 256],
                            ps_at[:, :w * 256].rearrange(
                                "p (w s) -> p w s", w=w),
                            den_b[:, :w * 256].rearrange(
                                "p (w s) -> p w s", w=w))
                    # all heads for this s-range done: ship + gather the slab.
                    # gpsimd is otherwise idle in B, so the collective's short
                    # input wait can't block anything.
                    nc.sync.dma_start(
                        cc_ains[g].rearrange("h p s -> p h s"),
                        attnT[:, :, g * 256:(g + 1) * 256])
                    nc.gpsimd.collective_compute(
                        "AllGather", ALU.bypass,
                        replica_groups=C.groups,
                        ins=[cc_ains[g].opt()], outs=[cc_aouts[g].opt()])

        # ---- phase C: output projection from gathered attnT ------------------
        with (
            tc.tile_pool(name="wop", bufs=1) as wop,
            tc.tile_pool(name="lhsp", bufs=2) as lhsp,
            tc.tile_pool(name="outp", bufs=2) as outp,
            tc.tile_pool(name="psO", bufs=3, space="PSUM") as psO,
        ):
            wo_sb = wop.tile([128, C.HQ, C.COLS], BF16)
            nc.sync.dma_start(wo_sb[:], wo_in[:])
            # slots are (replica r, local head h) r-major; global head = slot
            col_ts = []
            c0 = 0
            while c0 < C.COLS:
                wdt = min(512, C.COLS - c0)
                col_ts.append((c0, wdt))
                c0 += wdt
            for sb in range(NT):
                gi, off = sb // 2, (sb % 2) * 128
                lhs = lhsp.tile([128, C.HQ, 128], BF16, tag="lhs")
                nc.sync.dma_start(
                    lhs[:],
                    cc_aouts[gi][:, :, :, off:off + 128].rearrange(
                        "r h p s -> p (r h) s"))
                orow = outp.tile([128, C.COLS], F32, tag="orow")
                for (c0, wdt) in col_ts:
                    ps_o = psO.tile([128, 512], F32, tag="o")
                    for slot in range(C.HQ):
                        nc.tensor.matmul(ps_o[:, :wdt], lhs[:, slot, :],
                                         wo_sb[:, slot, c0:c0 + wdt],
                                         start=(slot == 0), stop=(slot == C.HQ - 1))
                    nc.vector.tensor_copy(orow[:, c0:c0 + wdt], ps_o[:, :wdt])
                nc.sync.dma_start(out_sh[sb * 128:(sb + 1) * 128, :], orow[:])


def build_program(C):
    nc = bacc.Bacc("TRN2", target_bir_lowering=False, debug=False,
                   num_devices=C.NC)
    io = {
        "xT16": nc.dram_tensor("xT16", [C.KC, 128, C.S], BF16, kind="ExternalInput").ap(),
        "wqkv": nc.dram_tensor("wqkv", [C.HCL, 128, C.KC, 128], BF16,
                               kind="ExternalInput").ap(),
        "wv_in": nc.dram_tensor("wv_in", [128, C.KC, C.KVL * 128], BF16,
                                kind="ExternalInput").ap(),
        "wo_in": nc.dram_tensor("wo_in", [128, C.HQ, C.COLS], BF16,
                                kind="ExternalInput").ap(),
        "cosT": nc.dram_tensor("cosT", [128, C.S], BF16, kind="ExternalInput").ap(),
        "sinT": nc.dram_tensor("sinT", [128, C.S], BF16, kind="ExternalInput").ap(),
        "masks": nc.dram_tensor("masks", [128, 4, 256], F32, kind="ExternalInput").ap(),
        "wnorm": nc.dram_tensor("wnorm", [128, C.HQL + C.KVL], F32,
                                kind="ExternalInput").ap(),
        "out_sh": nc.dram_tensor("out_sh", [C.S, C.COLS], F32,
                                 kind="ExternalOutput").ap(),
    }
    with tile.TileContext(nc) as tc:
        attention_tile_kernel(tc, C, io)
    nc.compile()
    return nc


def make_masks(mask_np, C):
    """4 mask tiles [t,s-pair] for offsets {0,1,WD,WD+1}; returns [128,4,256] f32."""
    S, WD, SW = C.S, C.WD, C.SW
    I0 = WD + 1

    def tileT(d):
        i, j = I0, I0 - d
        if 0 <= j < C.NT:
            blk = np.array(mask_np[i * 128:(i + 1) * 128, j * 128:(j + 1) * 128],
                           dtype=np.float64)
        else:
            blk = np.full((128, 128), -np.inf)
        s_idx = np.arange(128)[:, None]
        t_idx = np.arange(128)[None, :]
        dist = 128 * d + s_idx - t_idx
        blk = np.where(dist > SW, -np.inf, blk)
        return np.maximum(blk.T, -1e30).astype(np.float32)   # [t, s]

    tiles = []
    for off in (0, 1, WD, WD + 1):
        dl, dr = off - 1, off
        tiles.append(np.concatenate([tileT(dl), tileT(dr)], axis=1))
    return np.ascontiguousarray(np.stack(tiles, axis=1))      # [128, 4, 256]


def make_core_inputs(inputs, C):
    x = np.asarray(inputs["x"], dtype=np.float32)
    wq = np.asarray(inputs["wq"], dtype=np.float32)
    wk = np.asarray(inputs["wk"], dtype=np.float32)
    wv = np.asarray(inputs["wv"], dtype=np.float32)
    wo = np.asarray(inputs["wo"], dtype=np.float32)
    qw = np.asarray(inputs["q_norm_weight"], dtype=np.float32)
    kw = np.asarray(inputs["k_norm_weight"], dtype=np.float32)
    ch = np.asarray(inputs["cos_half"], dtype=np.float32)
    sh = np.asarray(inputs["sin_half"], dtype=np.float32)
    mask = np.asarray(inputs["mask"], dtype=np.float32)
    assert int(inputs.get("start_pos", 0) or 0) == 0

    cosT = np.ascontiguousarray(
        np.concatenate([ch.T, ch.T], axis=0)).astype(BF16_NP)
    sinT = np.ascontiguousarray(
        np.concatenate([-sh.T, sh.T], axis=0)).astype(BF16_NP)
    masks = make_masks(mask, C)
    KC, HQL, KVL = C.KC, C.HQL, C.KVL

    xT_cache = {}
    for b in range(C.DP):
        xT_cache[b] = np.ascontiguousarray(x[b].T).astype(BF16_NP).reshape(
            C.KC, 128, C.S)
    in_maps = []
    for c in range(C.NC):
        b, q4 = c // C.TP, c % C.TP
        x16 = xT_cache[b]
        wq_s = wq[:, 128 * HQL * q4:128 * HQL * (q4 + 1)]
        wk_s = wk[:, 128 * KVL * q4:128 * KVL * (q4 + 1)]
        wv_s = wv[:, 128 * KVL * q4:128 * KVL * (q4 + 1)]
        wqk = np.concatenate([wq_s, wk_s], axis=1).astype(BF16_NP)
        # [HCL, 128, KC, 128]: per chain, contraction-partition-major
        wqkv_pre = np.ascontiguousarray(
            wqk.reshape(KC, 128, C.HCL, 128).transpose(2, 1, 0, 3))
        wv_pre = np.ascontiguousarray(
            wv_s.astype(BF16_NP).reshape(KC, 128, KVL * 128).transpose(1, 0, 2))
        wo_s = wo[:, C.COLS * q4:C.COLS * (q4 + 1)].astype(BF16_NP)
        wo_pre = np.ascontiguousarray(
            wo_s.reshape(C.HQ, 128, C.COLS).transpose(1, 0, 2))
        wn = np.zeros((128, HQL + KVL), dtype=np.float32)
        for hc in range(HQL):
            g = HQL * q4 + hc
            wn[:, hc] = qw[128 * g:128 * (g + 1)]
        for j in range(KVL):
            g = KVL * q4 + j
            wn[:, HQL + j] = kw[128 * g:128 * (g + 1)]
        in_maps.append({"xT16": x16, "wqkv": wqkv_pre, "wv_in": wv_pre,
                        "wo_in": wo_pre, "cosT": cosT, "sinT": sinT,
                        "masks": masks, "wnorm": wn})
    return in_maps


_CACHED = {}


def run(inputs, C=None, trace=False, stitch=None, trace_cores=None):
    C = C or Cfg()
    key = (C.S, C.DIM, C.HQ, C.HKV, C.TP, C.DP, C.SW)
    if key not in _CACHED:
        _CACHED[key] = build_program(C)
    nc = _CACHED[key]
    in_maps = make_core_inputs(inputs, C)
    if stitch is None:
        stitch = trace
    if trace and trace_cores is None:
        trace_cores = list(range(C.NC))
    res = bass_utils.run_bass_kernel_spmd(
        nc, in_maps, core_ids=list(range(C.NC)), trace=trace,
        stitch_traces=stitch, trace_cores=trace_cores if trace else None)
    out = np.empty((C.DP, C.S, C.DIM), dtype=np.float32)
    for c in range(C.NC):
        b, q4 = c // C.TP, c % C.TP
        out[b, :, C.COLS * q4:C.COLS * (q4 + 1)] = res.results[c]["out_sh"]
    return out, res


def kernel(**inputs) -> np.ndarray:
    out, _ = run(inputs)
    return out


# revision 41
# speedup vs baseline: 1.1571x; 1.0080x over previous
"""Sparse (sliding-window) GQA attention prefill kernel for 8 Trainium2 cores.

Sharding: TP=4 over KV heads (2 KV heads + 10 Q heads per core) x DP=2 over
batch. Core c: batch = c // 4, shard q4 = c % 4.

Device program (SPMD, identical on all cores; per-core data via inputs):
  A1: xqT/xkT projections in transposed layout ([head_dim, seq]); sum-of-squares
      for the global RMS norm via Square + ones-matmul; per-s-tile AllReduce of
      the norm partials within each batch group (overlaps A1 compute).
  A2: V projection in natural layout ([seq, head_dim]).
  R:  norm rows -> rope tables (bf16, norm factor folded in); batched rope over
      all local heads per 512-slice (vector for q, gpsimd for k), overlapping A2.
  B:  per (head-pair chunk, 2-query-tile group): scoresT = K^T-chunk.T @ qT in
      the sliding band, mask add, exp, ones-matmul denominator, P^T @ V
      accumulation. Software-pipelined: the denominator/PV matmuls for step j
      are emitted after the scores matmul of step j+1 so the PE never waits on
      the exp. Divide on evacuation via broadcast + reciprocal on [128, .].
      attnT shipped in s-halves: AllGather per (chunk, half), issued late so the
      collective's input wait never blocks the gpsimd queue.
  C:  out = attnT.T @ wo col-shard; first s-half rows start as soon as the
      half-0 gathers land.
"""

import sys
import numpy as np

for _p in ("/opt/trn_rl_repo", "/root/.axon_site/_ro/trn_rl_repo"):
    if _p not in sys.path:
        sys.path.insert(0, _p)

import ml_dtypes

import concourse.bass as bass
import concourse.tile as tile
from concourse import bacc, mybir
from concourse import bass_utils

F32 = mybir.dt.float32
BF16 = mybir.dt.bfloat16
BF16_NP = ml_dtypes.bfloat16
AF = mybir.ActivationFunctionType
ALU = mybir.AluOpType


class Cfg:
    def __init__(self, S=2048, DIM=5120, HQ=40, HKV=8, TP=4, DP=2, SW=1024,
                 MSCALE=1.2079441541679836, EPS=1e-6):
        self.S, self.DIM, self.HQ, self.HKV = S, DIM, HQ, HKV
        self.TP, self.DP, self.SW = TP, DP, SW
        self.MSCALE, self.EPS = MSCALE, EPS
        self.D = 128
        self.NC = TP * DP
        self.HQL = HQ // TP          # local q heads
        self.KVL = HKV // TP         # local kv heads
        self.REP = HQ // HKV
        self.KC = DIM // 128         # contraction chunks
        self.NT = S // 128           # seq tiles
        self.G = self.NT // 2        # 2-query-tile groups
        self.WD = SW // 128          # window in tiles
        self.COLS = DIM // TP        # output column shard
        self.HCL = self.HQL + self.KVL  # projection chains with transposed out
        self.NST = S // 512          # 512-wide s-tiles (phase A1)
        self.NST2 = S // 256         # 256-wide s-tiles (phase A2)
        self.CQ = self.D ** -0.5 * MSCALE
        assert self.WD >= 2 and self.NT > self.WD + 1 and self.NT % 2 == 0
        self.groups = [[b * TP + r for r in range(TP)] for b in range(DP)]


def head_chunks(C):
    """Per-kv head pair chunks: [(kv, [h0,h1]), (kv, [h2,h3]), (kv, [h4])...]"""
    out = []
    per = C.HQL // C.KVL
    for kv in range(C.KVL):
        hs = list(range(kv * per, (kv + 1) * per))
        i = 0
        while i < len(hs):
            out.append((kv, hs[i:i + 2]))
            i += 2
    return out


def attention_tile_kernel(tc, C, io):
    nc = tc.nc
    S, KC, HQL, KVL, NT, G, WD = C.S, C.KC, C.HQL, C.KVL, C.NT, C.G, C.WD
    H2 = S // 2
    xT16, wqkv, wv_in, wo_in = io["xT16"], io["wqkv"], io["wv_in"], io["wo_in"]
    cosT_in, sinT_in, masks_in, wnorm_in = io["cosT"], io["sinT"], io["masks"], io["wnorm"]
    out_sh = io["out_sh"]
    chunks = head_chunks(C)

    from contextlib import ExitStack
    ctx = ExitStack()
    with ctx:
        singles = ctx.enter_context(tc.tile_pool(name="singles", bufs=1))
        dramcc = ctx.enter_context(tc.tile_pool(name="dramcc", bufs=1, space="DRAM"))

        ones16 = singles.tile([128, 1], BF16)
        nc.vector.memset(ones16[:], 1.0)
        ones128 = singles.tile([128, 128], BF16)
        nc.vector.memset(ones128[:], 1.0)
        wnorm_sb = singles.tile([128, HQL + KVL], F32)
        nc.sync.dma_start(wnorm_sb[:], wnorm_in[:])
        cq2 = C.CQ * C.CQ
        eps_q = singles.tile([128, 1], F32)
        nc.vector.memset(eps_q[:], C.EPS / cq2)
        eps_k = singles.tile([128, 1], F32)
        nc.vector.memset(eps_k[:], C.EPS)

        cc_nins = [dramcc.tile([1, 1024], F32, name=f"ccni{st}")
                   for st in range(C.NST)]
        cc_nouts = [dramcc.tile([1, 1024], F32, name=f"ccno{st}")
                    for st in range(C.NST)]
        # one gather per 512-wide s-range (phase B runs g-outer), so phase C
        # consumes s-slabs progressively and never waits on the last
        # collective; 4 gathers keeps the ~40us fixed CC cost per collective
        # well under phase B's span
        NGA = G // 2
        cc_ains = [dramcc.tile([HQL, 128, 512], BF16, name=f"ccag{gi}")
                   for gi in range(NGA)]
        cc_aouts = [dramcc.tile([C.TP, HQL, 128, 512], BF16,
                                name=f"ccaog{gi}") for gi in range(NGA)]

        wvp = ctx.enter_context(tc.tile_pool(name="wvp", bufs=1))
        wv_sb = wvp.tile([128, KC, KVL * 128], BF16)
        nc.gpsimd.dma_start(wv_sb[:], wv_in[:])

        with (
            tc.tile_pool(name="xqp", bufs=1) as xq_pool,
            tc.tile_pool(name="xkp", bufs=1) as xk_pool,
            tc.tile_pool(name="vp", bufs=1) as v_pool,
        ):
            xq_sb = xq_pool.tile([128, HQL, S], BF16)
            xk_sb = xk_pool.tile([128, KVL, S], BF16)
            v_sb = v_pool.tile([128, NT, KVL, 128], BF16)

            # ---- phase A: q/k projections (transposed out) + norm
            # partials + V projection, all per 512-wide s-tile. V reuses the
            # same xt tiles as stationary, so x is loaded exactly once.
            # Rope chains are emitted with a 2-tile lag so each slice's
            # AllReduce has completed long before its chain runs; slices 0-1
            # rope on the vector engine during A itself.
            KCH = KC // 2
            with (
                tc.tile_pool(name="xt1", bufs=2) as xt1,
                tc.tile_pool(name="wst", bufs=2) as wst,
                tc.tile_pool(name="sqp", bufs=2) as sqp,
                tc.tile_pool(name="trow", bufs=1) as trow,
                tc.tile_pool(name="tabsQ", bufs=1) as tabs_q,
                tc.tile_pool(name="rowsp", bufs=1) as rowsp,
                tc.tile_pool(name="ropep", bufs=1) as ropep,
                tc.tile_pool(name="psA", bufs=3, space="PSUM") as psA,
                tc.tile_pool(name="psN", bufs=1, space="PSUM") as psN,
                tc.tile_pool(name="psV", bufs=3, space="PSUM") as psV,
            ):

                def emit_rchain(st):
                    sl = slice(st * 512, (st + 1) * 512)
                    # broadcast raw partial sums, then r = exp(-0.5*ln(ax+b))
                    rowraw = rowsp.tile([1, 1024], F32, tag="rowraw")
                    nc.gpsimd.dma_start(rowraw[:], cc_nouts[st][:])
                    rq_b = rowsp.tile([128, 512], F32, tag="rqb")
                    rk_b = rowsp.tile([128, 512], F32, tag="rkb")
                    nc.gpsimd.partition_broadcast(rk_b[:],
                                                  rowraw[0:1, 512:1024])
                    nc.gpsimd.partition_broadcast(rq_b[:],
                                                  rowraw[0:1, 0:512])
                    nc.scalar.activation(rk_b[:], rk_b[:], AF.Ln,
                                         scale=1.0 / (C.HKV * 128),
                                         bias=eps_k[:])
                    nc.scalar.activation(rk_b[:], rk_b[:], AF.Exp, scale=-0.5)
                    nc.scalar.activation(rq_b[:], rq_b[:], AF.Ln,
                                         scale=1.0 / (C.DIM * cq2),
                                         bias=eps_q[:])
                    nc.scalar.activation(rq_b[:], rq_b[:], AF.Exp, scale=-0.5)
                    cosq = tabs_q.tile([128, 512], BF16, tag="cosq")
                    sinq = tabs_q.tile([128, 512], BF16, tag="sinq")
                    cosk = tabs_q.tile([128, 512], BF16, tag="cosk")
                    sink = tabs_q.tile([128, 512], BF16, tag="sink")
                    nc.gpsimd.dma_start(cosk[:], cosT_in[:, sl])
                    nc.gpsimd.dma_start(sink[:], sinT_in[:, sl])
                    nc.gpsimd.dma_start(cosq[:], cosT_in[:, sl])
                    nc.gpsimd.dma_start(sinq[:], sinT_in[:, sl])
                    nc.vector.tensor_mul(cosk[:], cosk[:], rk_b[:])
                    nc.vector.tensor_mul(sink[:], sink[:], rk_b[:])
                    nc.vector.tensor_mul(cosq[:], cosq[:], rq_b[:])
                    nc.vector.tensor_mul(sinq[:], sinq[:], rq_b[:])
                    rotk = ropep.tile([128, KVL, 512], BF16, tag="rotk")
                    rotq = ropep.tile([128, HQL, 512], BF16, tag="rotq")
                    nc.gpsimd.dma_start(rotk[0:64], xk_sb[64:128, :, sl])
                    nc.gpsimd.dma_start(rotk[64:128], xk_sb[0:64, :, sl])
                    nc.gpsimd.dma_start(rotq[0:64], xq_sb[64:128, :, sl])
                    nc.gpsimd.dma_start(rotq[64:128], xq_sb[0:64, :, sl])
                    # fully in-place on vector: rot *= sin; x *= cos; x += rot
                    nc.vector.tensor_mul(
                        rotk[:], rotk[:],
                        sink[:, None, :].to_broadcast((128, KVL, 512)))
                    nc.vector.tensor_mul(
                        xk_sb[:, :, sl], xk_sb[:, :, sl],
                        cosk[:, None, :].to_broadcast((128, KVL, 512)))
                    nc.vector.tensor_add(xk_sb[:, :, sl], xk_sb[:, :, sl],
                                         rotk[:])
                    nc.vector.tensor_mul(
                        rotq[:], rotq[:],
                        sinq[:, None, :].to_broadcast((128, HQL, 512)))
                    nc.vector.tensor_mul(
                        xq_sb[:, :, sl], xq_sb[:, :, sl],
                        cosq[:, None, :].to_broadcast((128, HQL, 512)))
                    nc.vector.tensor_add(xq_sb[:, :, sl], xq_sb[:, :, sl],
                                         rotq[:])

                for st in range(C.NST):
                    if st >= 2:
                        emit_rchain(st - 2)
                    s0 = st * 512
                    xt_a = xt1.tile([128, KCH, 512], BF16, tag="xta")
                    xt_b = xt1.tile([128, KCH, 512], BF16, tag="xtb")
                    nc.sync.dma_start(
                        xt_a[:],
                        xT16[:KCH, :, s0:s0 + 512].rearrange("kc p s -> p kc s"))
                    nc.sync.dma_start(
                        xt_b[:],
                        xT16[KCH:, :, s0:s0 + 512].rearrange("kc p s -> p kc s"))
                    ps_nq = psN.tile([1, 512], F32, tag="nq")
                    ps_nk = psN.tile([1, 512], F32, tag="nk")
                    for hc in range(C.HCL):
                        w_sb = wst.tile([128, KC, 128], BF16, tag="w")
                        nc.sync.dma_start(w_sb[:], wqkv[hc])
                        ps = psA.tile([128, 512], F32, tag="proj")
                        for kc in range(KC):
                            xsrc = xt_a if kc < KCH else xt_b
                            nc.tensor.matmul(ps[:], w_sb[:, kc, :],
                                             xsrc[:, kc % KCH, :],
                                             start=(kc == 0), stop=(kc == KC - 1))
                        if hc < HQL:
                            dest = xq_sb[:, hc, s0:s0 + 512]
                        else:
                            dest = xk_sb[:, hc - HQL, s0:s0 + 512]
                        nc.vector.tensor_scalar_mul(dest, ps[:],
                                                    wnorm_sb[:, hc:hc + 1])
                        sq = sqp.tile([128, 512], BF16, tag="sq")
                        nc.scalar.activation(sq[:], ps[:], AF.Square)
                        tgt = ps_nq if hc < HQL else ps_nk
                        first = (hc == 0) or (hc == HQL)
                        last = (hc == HQL - 1) or (hc == C.HCL - 1)
                        nc.tensor.matmul(tgt[:], ones16[:], sq[:],
                                         start=first, stop=last)
                    rq_t = trow.tile([1, 512], F32, tag="rq")
                    rk_t = trow.tile([1, 512], F32, tag="rk")
                    nc.vector.tensor_copy(rq_t[:], ps_nq[:])
                    nc.vector.tensor_copy(rk_t[:], ps_nk[:])
                    nc.sync.dma_start(cc_nins[st][0:1, 0:512], rq_t[:])
                    nc.sync.dma_start(cc_nins[st][0:1, 512:1024], rk_t[:])
                    # per-s-tile AllReduce of norm partials: overlaps A compute
                    nc.gpsimd.collective_compute(
                        "AllReduce", ALU.add, replica_groups=C.groups,
                        ins=[cc_nins[st].opt()], outs=[cc_nouts[st].opt()])
                    # V projection for this s-tile, x chunks as stationary
                    for tc4 in range(4):
                        tt = st * 4 + tc4
                        psv = psV.tile([128, KVL * 128], F32, tag="v")
                        for kc in range(KC):
                            xsrc = xt_a if kc < KCH else xt_b
                            nc.tensor.matmul(
                                psv[:],
                                xsrc[:, kc % KCH,
                                     tc4 * 128:(tc4 + 1) * 128],
                                wv_sb[:, kc, :],
                                start=(kc == 0), stop=(kc == KC - 1))
                        nc.scalar.copy(v_sb[:, tt, :, :], psv[:])

                emit_rchain(C.NST - 2)
                emit_rchain(C.NST - 1)

            # ---- phase B: banded attention, software-pipelined --------------
            with (
                tc.tile_pool(name="attnp", bufs=1) as attnp,
                tc.tile_pool(name="maskp", bufs=1) as maskp,
                tc.tile_pool(name="expp", bufs=4) as expp,
                tc.tile_pool(name="bmisc", bufs=3) as bmisc,
                tc.tile_pool(name="psSC", bufs=2, space="PSUM") as psSC,
                tc.tile_pool(name="psAT", bufs=2, space="PSUM") as psAT,
                tc.tile_pool(name="psDN", bufs=2, space="PSUM") as psDN,
            ):
                attnT = attnp.tile([128, HQL, S], BF16)
                masks_sb = maskp.tile([128, 4, 256], F32)
                nc.sync.dma_start(masks_sb[:], masks_in[:])
                off2m = {0: 0, 1: 1, WD: 2, WD + 1: 3}

                for g in range(G):
                    jlo, jhi = max(0, 2 * g - WD), 2 * g + 1
                    npairs = (jhi - jlo + 1) // 2
                    for ci, (kv, hs) in enumerate(chunks):
                        w = len(hs)
                        h0 = hs[0]
                        ps_at = psAT.tile([128, 512], F32, tag="at")
                        # ones128 stationary -> denominator lands pre-broadcast
                        # across all 128 partitions (and avoids the 1-wide
                        # stationary pipeline penalty)
                        ps_dn = psDN.tile([128, 512], F32, tag="dn")
                        pend = []  # (ex slice, j) waiting for dn/at emission

                        def drain():
                            for exp_, jp in pend:
                                nc.tensor.matmul(ps_dn[:, :w * 256],
                                                 ones128[:],
                                                 exp_, start=(jp == jlo),
                                                 stop=(jp == jhi))
                                nc.tensor.matmul(ps_at[:, :w * 256],
                                                 v_sb[:, jp, kv, :],
                                                 exp_, start=(jp == jlo),
                                                 stop=(jp == jhi))
                            pend.clear()

                        for p in range(npairs):
                            j0 = jlo + 2 * p
                            ps2 = psSC.tile([128, 1024], F32, tag="sc")
                            for dj in range(2):
                                j = j0 + dj
                                o = dj * 512
                                nc.tensor.matmul(
                                    ps2[:, o:o + w * 256].rearrange(
                                        "p (w s) -> p w s", w=w),
                                    xk_sb[:, kv, j * 128:(j + 1) * 128],
                                    xq_sb[:, h0:h0 + w, g * 256:(g + 1) * 256],
                                    start=True, stop=True)
                            # consume the previous pair while this pair's exp
                            # runs -> the PE never waits on the scalar engine
                            drain()
                            for dj in range(2):
                                j = j0 + dj
                                m = off2m.get(jhi - j)
                                if m is not None:
                                    o = dj * 512
                                    nc.vector.tensor_add(
                                        ps2[:, o:o + w * 256].rearrange(
                                            "p (w s) -> p w s", w=w),
                                        ps2[:, o:o + w * 256].rearrange(
                                            "p (w s) -> p w s", w=w),
                                        masks_sb[:, m, None, :].to_broadcast(
                                            (128, w, 256)))
                            ex2 = expp.tile([128, 1024], BF16, tag="ex")
                            nc.scalar.activation(
                                ex2.rearrange("p (j s) -> p j s",
                                              j=2)[:, :, :w * 256],
                                ps2.rearrange("p (j s) -> p j s",
                                              j=2)[:, :, :w * 256],
                                AF.Exp)
                            pend.append((ex2[:, 0:w * 256], j0))
                            pend.append((ex2[:, 512:512 + w * 256], j0 + 1))
                        drain()
                        den_b = bmisc.tile([128, 512], F32, tag="denb")
                        nc.vector.reciprocal_approx_fast(
                            out=den_b[:, :w * 256], in_=ps_dn[:, :w * 256])
                        nc.vector.tensor_mul(
                            attnT[:, h0:h0 + w, g * 256:(g + 1) * 256],
                            ps_at[:, :w * 256].rearrange(
                                "p (w s) -> p w s", w=w),
                            den_b[:, :w * 256].rearrange(
                                "p (w s) -> p w s", w=w))
                    # all heads for this s-range done: ship + gather the slab
                    # every second g. gpsimd is otherwise idle in B, so the
                    # collective's short input wait can't block anything.
                    if g % 2 == 1:
                        gi = g // 2
                        nc.sync.dma_start(
                            cc_ains[gi].rearrange("h p s -> p h s"),
                            attnT[:, :, (g - 1) * 256:(g + 1) * 256])
                        nc.gpsimd.collective_compute(
                            "AllGather", ALU.bypass,
                            replica_groups=C.groups,
                            ins=[cc_ains[gi].opt()], outs=[cc_aouts[gi].opt()])

        # ---- phase C: output projection from gathered attnT ------------------
        with (
            tc.tile_pool(name="wop", bufs=1) as wop,
            tc.tile_pool(name="lhsp", bufs=2) as lhsp,
            tc.tile_pool(name="outp", bufs=2) as outp,
            tc.tile_pool(name="psO", bufs=3, space="PSUM") as psO,
        ):
            wo_sb = wop.tile([128, C.HQ, C.COLS], BF16)
            nc.sync.dma_start(wo_sb[:], wo_in[:])
            # slots are (replica r, local head h) r-major; global head = slot
            col_ts = []
            c0 = 0
            while c0 < C.COLS:
                wdt = min(512, C.COLS - c0)
                col_ts.append((c0, wdt))
                c0 += wdt
            for sb in range(NT):
                gi, off = sb // 4, (sb % 4) * 128
                lhs = lhsp.tile([128, C.HQ, 128], BF16, tag="lhs")
                nc.sync.dma_start(
                    lhs[:],
                    cc_aouts[gi][:, :, :, off:off + 128].rearrange(
                        "r h p s -> p (r h) s"))
                orow = outp.tile([128, C.COLS], F32, tag="orow")
                for (c0, wdt) in col_ts:
                    ps_o = psO.tile([128, 512], F32, tag="o")
                    for slot in range(C.HQ):
                        nc.tensor.matmul(ps_o[:, :wdt], lhs[:, slot, :],
                                         wo_sb[:, slot, c0:c0 + wdt],
                                         start=(slot == 0), stop=(slot == C.HQ - 1))
                    nc.vector.tensor_copy(orow[:, c0:c0 + wdt], ps_o[:, :wdt])
                nc.sync.dma_start(out_sh[sb * 128:(sb + 1) * 128, :], orow[:])


def build_program(C):
    nc = bacc.Bacc("TRN2", target_bir_lowering=False, debug=False,
                   num_devices=C.NC)
    io = {
        "xT16": nc.dram_tensor("xT16", [C.KC, 128, C.S], BF16, kind="ExternalInput").ap(),
        "wqkv": nc.dram_tensor("wqkv", [C.HCL, 128, C.KC, 128], BF16,
                               kind="ExternalInput").ap(),
        "wv_in": nc.dram_tensor("wv_in", [128, C.KC, C.KVL * 128], BF16,
                                kind="ExternalInput").ap(),
        "wo_in": nc.dram_tensor("wo_in", [128, C.HQ, C.COLS], BF16,
                                kind="ExternalInput").ap(),
        "cosT": nc.dram_tensor("cosT", [128, C.S], BF16, kind="ExternalInput").ap(),
        "sinT": nc.dram_tensor("sinT", [128, C.S], BF16, kind="ExternalInput").ap(),
        "masks": nc.dram_tensor("masks", [128, 4, 256], F32, kind="ExternalInput").ap(),
        "wnorm": nc.dram_tensor("wnorm", [128, C.HQL + C.KVL], F32,
                                kind="ExternalInput").ap(),
        "out_sh": nc.dram_tensor("out_sh", [C.S, C.COLS], F32,
                                 kind="ExternalOutput").ap(),
    }
    with tile.TileContext(nc) as tc:
        attention_tile_kernel(tc, C, io)
    nc.compile()
    return nc


def make_masks(mask_np, C):
    """4 mask tiles [t,s-pair] for offsets {0,1,WD,WD+1}; returns [128,4,256] f32."""
    S, WD, SW = C.S, C.WD, C.SW
    I0 = WD + 1

    def tileT(d):
        i, j = I0, I0 - d
        if 0 <= j < C.NT:
            blk = np.array(mask_np[i * 128:(i + 1) * 128, j * 128:(j + 1) * 128],
                           dtype=np.float64)
        else:
            blk = np.full((128, 128), -np.inf)
        s_idx = np.arange(128)[:, None]
        t_idx = np.arange(128)[None, :]
        dist = 128 * d + s_idx - t_idx
        blk = np.where(dist > SW, -np.inf, blk)
        return np.maximum(blk.T, -1e30).astype(np.float32)   # [t, s]

    tiles = []
    for off in (0, 1, WD, WD + 1):
        dl, dr = off - 1, off
        tiles.append(np.concatenate([tileT(dl), tileT(dr)], axis=1))
    return np.ascontiguousarray(np.stack(tiles, axis=1))      # [128, 4, 256]


def make_core_inputs(inputs, C):
    x = np.asarray(inputs["x"], dtype=np.float32)
    wq = np.asarray(inputs["wq"], dtype=np.float32)
    wk = np.asarray(inputs["wk"], dtype=np.float32)
    wv = np.asarray(inputs["wv"], dtype=np.float32)
    wo = np.asarray(inputs["wo"], dtype=np.float32)
    qw = np.asarray(inputs["q_norm_weight"], dtype=np.float32)
    kw = np.asarray(inputs["k_norm_weight"], dtype=np.float32)
    ch = np.asarray(inputs["cos_half"], dtype=np.float32)
    sh = np.asarray(inputs["sin_half"], dtype=np.float32)
    mask = np.asarray(inputs["mask"], dtype=np.float32)
    assert int(inputs.get("start_pos", 0) or 0) == 0

    cosT = np.ascontiguousarray(
        np.concatenate([ch.T, ch.T], axis=0)).astype(BF16_NP)
    sinT = np.ascontiguousarray(
        np.concatenate([-sh.T, sh.T], axis=0)).astype(BF16_NP)
    masks = make_masks(mask, C)
    KC, HQL, KVL = C.KC, C.HQL, C.KVL

    xT_cache = {}
    for b in range(C.DP):
        xT_cache[b] = np.ascontiguousarray(x[b].T).astype(BF16_NP).reshape(
            C.KC, 128, C.S)
    in_maps = []
    for c in range(C.NC):
        b, q4 = c // C.TP, c % C.TP
        x16 = xT_cache[b]
        wq_s = wq[:, 128 * HQL * q4:128 * HQL * (q4 + 1)]
        wk_s = wk[:, 128 * KVL * q4:128 * KVL * (q4 + 1)]
        wv_s = wv[:, 128 * KVL * q4:128 * KVL * (q4 + 1)]
        wqk = np.concatenate([wq_s, wk_s], axis=1).astype(BF16_NP)
        # [HCL, 128, KC, 128]: per chain, contraction-partition-major
        wqkv_pre = np.ascontiguousarray(
            wqk.reshape(KC, 128, C.HCL, 128).transpose(2, 1, 0, 3))
        wv_pre = np.ascontiguousarray(
            wv_s.astype(BF16_NP).reshape(KC, 128, KVL * 128).transpose(1, 0, 2))
        wo_s = wo[:, C.COLS * q4:C.COLS * (q4 + 1)].astype(BF16_NP)
        wo_pre = np.ascontiguousarray(
            wo_s.reshape(C.HQ, 128, C.COLS).transpose(1, 0, 2))
        wn = np.zeros((128, HQL + KVL), dtype=np.float32)
        for hc in range(HQL):
            g = HQL * q4 + hc
            wn[:, hc] = qw[128 * g:128 * (g + 1)]
        for j in range(KVL):
            g = KVL * q4 + j
            wn[:, HQL + j] = kw[128 * g:128 * (g + 1)]
        in_maps.append({"xT16": x16, "wqkv": wqkv_pre, "wv_in": wv_pre,
                        "wo_in": wo_pre, "cosT": cosT, "sinT": sinT,
                        "masks": masks, "wnorm": wn})
    return in_maps


_CACHED = {}


def run(inputs, C=None, trace=False, stitch=None, trace_cores=None):
    C = C or Cfg()
    key = (C.S, C.DIM, C.HQ, C.HKV, C.TP, C.DP, C.SW)
    if key not in _CACHED:
        _CACHED[key] = build_program(C)
    nc = _CACHED[key]
    in_maps = make_core_inputs(inputs, C)
    if stitch is None:
        stitch = trace
    if trace and trace_cores is None:
        trace_cores = list(range(C.NC))
    res = bass_utils.run_bass_kernel_spmd(
        nc, in_maps, core_ids=list(range(C.NC)), trace=trace,
        stitch_traces=stitch, trace_cores=trace_cores if trace else None)
    out = np.empty((C.DP, C.S, C.DIM), dtype=np.float32)
    for c in range(C.NC):
        b, q4 = c // C.TP, c % C.TP
        out[b, :, C.COLS * q4:C.COLS * (q4 + 1)] = res.results[c]["out_sh"]
    return out, res


def kernel(**inputs) -> np.ndarray:
    out, _ = run(inputs)
    return out
